# revision 7
# baseline (speedup 1.0000x reference)
# Multi-head attention (B=4, S=2048, D=512, H=8) on 8 Trainium2 cores.
#
# Sharding: core c = (batch b=c//2, head-group g=c%2, 4 heads each). Every core
# runs the identical program (SPMD) on its own slice; partial w_o outputs of the
# two head-groups of a batch are summed on the host (+ b_o).
#
# Device dataflow keeps every activation transposed ([feature, token]) so no
# on-device transposes are needed:
#   QT = w_q_g @ qT + b  (PE, din on partitions)        [256, 2048]
#   KT = (w_k_g/8) @ kT + b/8                            [256, 2048]
#   V  = natural [keys, dout] via lhsT = vT chunks       [2048, 4, 65] (+ones col)
#   scores^T[k, q] = K Q^T  (lhsT = KT slice)            per (qc=512, kc=128)
#   E^T = exp(scores^T + causal window mask)             ACT, merged head-pair
#   ctx^T/l = V_aug^T E^T   (m=65: row 64 = l[q])        PSUM accumulate over kc
#   out^T_partial = w_oT_g ctx^T                         [512, 2048] -> DRAM
#
# All matmul operands are float32r (full-rate fp32 on the PE; producers round
# on write). Resident tensors are split into per-512-chunk tiles so phases
# overlap, and each q-chunk's output projection is emitted inside the next
# chunk's attention loop to keep the PE stream dense (HAM stays warm).
import os
import sys

import numpy as np

B, S, D, H = 4, 2048, 512, 8
DK = D // H          # 64
P = 128
NCORES = 8
HG = 2               # head groups (cores per batch)
DH = D // HG         # 256 out dims per core
LH = H // HG         # 4 local heads
QCS = 512            # q/key chunk size
NQC = S // QCS       # 4
KCS = 128            # key tile size (scores psum partition dim)
NEG = -1e9

# "f32r" (replicated fp32, 1 cyc/row at free>=256) or "f32" (exact, 4 cyc/row)
MM_DT = os.environ.get("KERNEL_MM_DT", "f32r")

_CACHE = {}


def _import_concourse():
    for p in ("/opt/trn_rl_repo", "/root/.axon_site/_ro/trn_rl_repo"):
        if os.path.isdir(p) and p not in sys.path:
            sys.path.append(p)
    import concourse.bass as bass          # noqa: F401
    import concourse.mybir as mybir        # noqa: F401
    import concourse.tile as tile          # noqa: F401
    from concourse import bacc             # noqa: F401
    return bass, mybir, tile


def build_nc():
    """Build the (single, shared-by-all-cores) Bass program."""
    if "nc" in _CACHE:
        return _CACHE["nc"]
    bass, mybir, tile = _import_concourse()
    from concourse import bacc
    from contextlib import ExitStack

    f32 = mybir.dt.float32
    fr = mybir.dt.float32r if MM_DT == "f32r" else f32
    Exp = mybir.ActivationFunctionType.Exp

    nc = bacc.Bacc("TRN2", target_bir_lowering=False, debug=False)

    qT = nc.dram_tensor("qT", [D, S], fr, kind="ExternalInput").ap()
    kT = nc.dram_tensor("kT", [D, S], fr, kind="ExternalInput").ap()
    vT = nc.dram_tensor("vT", [D, S], fr, kind="ExternalInput").ap()
    wqT = nc.dram_tensor("wqT", [D, DH], fr, kind="ExternalInput").ap()
    wkT = nc.dram_tensor("wkT", [D, DH], fr, kind="ExternalInput").ap()
    wvT = nc.dram_tensor("wvT", [D, DH], fr, kind="ExternalInput").ap()
    woT = nc.dram_tensor("woT", [DH, D], fr, kind="ExternalInput").ap()
    bq = nc.dram_tensor("bq", [2, P], f32, kind="ExternalInput").ap()
    bk = nc.dram_tensor("bk", [2, P], f32, kind="ExternalInput").ap()
    bv = nc.dram_tensor("bv", [P, DH], f32, kind="ExternalInput").ap()
    mtri = nc.dram_tensor("mtri", [P, P], f32, kind="ExternalInput").ap()
    onesd = nc.dram_tensor("onesd", [P, DK], f32, kind="ExternalInput").ap()
    outT = nc.dram_tensor("outT", [D, S], f32, kind="ExternalOutput").ap()

    with tile.TileContext(nc) as tc, ExitStack() as ctx:
        wpool = ctx.enter_context(tc.tile_pool(name="weights", bufs=1))
        res = ctx.enter_context(tc.tile_pool(name="resident", bufs=1))
        opool = ctx.enter_context(tc.tile_pool(name="ost", bufs=4))

        wq_sb = wpool.tile([P, 4, DH], fr, tag="wq")
        nc.sync.dma_start(wq_sb[:], wqT.rearrange("(c p) m -> p c m", p=P))
        wk_sb = wpool.tile([P, 4, DH], fr, tag="wk")
        nc.sync.dma_start(wk_sb[:], wkT.rearrange("(c p) m -> p c m", p=P))
        wv_sb = wpool.tile([P, 4, DH], fr, tag="wv")
        nc.sync.dma_start(wv_sb[:], wvT.rearrange("(c p) m -> p c m", p=P))
        wo_sb = wpool.tile([P, 2, D], fr, tag="wo")
        nc.sync.dma_start(wo_sb[:], woT.rearrange("(c p) m -> p c m", p=P))
        bq_sb = wpool.tile([P, 2], f32, tag="bq")
        nc.sync.dma_start(bq_sb[:], bq.rearrange("c p -> p c"))
        bk_sb = wpool.tile([P, 2], f32, tag="bk")
        nc.sync.dma_start(bk_sb[:], bk.rearrange("c p -> p c"))
        bv_sb = wpool.tile([P, DH], f32, tag="bv")
        nc.sync.dma_start(bv_sb[:], bv[:])
        mt_sb = wpool.tile([P, P], f32, tag="mtri")
        nc.sync.dma_start(mt_sb[:], mtri[:])
        ones_sb = wpool.tile([P, DK], f32, tag="ones")
        nc.sync.dma_start(ones_sb[:], onesd[:])

        # per-512-chunk resident tiles -> fine-grained cross-phase deps
        QTs = [res.tile([P, 2, QCS], fr, tag=f"QT{i}", name=f"QT{i}") for i in range(NQC)]
        KTs = [res.tile([P, 2, QCS], fr, tag=f"KT{i}", name=f"KT{i}") for i in range(NQC)]
        Vgs = [
            res.tile([P, 4, LH, DK + 1], fr, tag=f"Vg{i}", name=f"Vg{i}")
            for i in range(NQC)
        ]
        CTs = [res.tile([P, 2, QCS], fr, tag=f"CT{i}", name=f"CT{i}") for i in range(NQC)]

        qT_r = qT.rearrange("(c p) f -> p c f", p=P)
        kT_r = kT.rearrange("(c p) f -> p c f", p=P)
        vT_r = vT.rearrange("(c p) f -> p c f", p=P)
        bv_r = bv_sb.rearrange("p (h d) -> p h d", h=LH)
        ones_r = ones_sb[:, 0 : 4 * LH].rearrange("p (a b) -> p a b", a=4)

        # ---- Unified pipeline ----
        # For i in 0..3: project 512-chunk i, then attention for q-chunk i
        # (causal: it only consumes chunks <= i), then the previous chunk's
        # output projection. Projections share the scores PSUM pool so the
        # whole kernel fits the 8 banks and the PE stream never breaks.
        with (
            tc.tile_pool(name="inq", bufs=2) as qpool,
            tc.tile_pool(name="ink", bufs=2) as kpool,
            tc.tile_pool(name="inv", bufs=2) as vpool,
            tc.tile_pool(name="et", bufs=6) as epool,
            tc.tile_pool(name="sc", bufs=2, space="PSUM") as scp,
            tc.tile_pool(name="cx", bufs=2, space="PSUM") as cxp,
            tc.tile_pool(name="ls", bufs=1) as lpool,
            tc.tile_pool(name="cbst", bufs=2) as cbpool,
        ):

            # Projections, output projections and attention all share one PE
            # instruction stream: proj/oproj matmul groups are interleaved as
            # "fillers" between attention iterations. A filler allocates from
            # the same "sc" PSUM rotation as the score tiles, so it only ever
            # waits on an exp already in flight (never on anything behind it
            # in the in-order PE queue). The PE therefore never has a
            # low-duty window and the HAM clock-gate stays at 2.4 GHz.
            def fetch_chunk(fc):
                sl = slice(fc * QCS, (fc + 1) * QCS)
                kch = kpool.tile([P, 4, QCS], fr, tag="kch", name=f"kch{fc}")
                nc.sync.dma_start(kch[:], kT_r[:, :, sl])
                qch = qpool.tile([P, 4, QCS], fr, tag="qch", name=f"qch{fc}")
                nc.sync.dma_start(qch[:], qT_r[:, :, sl])
                vch = vpool.tile([P, 4, QCS], fr, tag="vch", name=f"vch{fc}")
                nc.sync.dma_start(vch[:], vT_r[:, :, sl])
                return qch, kch, vch

            def proj_groups(fc, qch, kch, vch):
                def g_q():
                    ps = scp.tile([P, 2, QCS], f32, tag="sc", name=f"psq{fc}")
                    for mo in range(2):
                        for c in range(4):
                            nc.tensor.matmul(
                                ps[:, mo, :], wq_sb[:, c, mo * P : (mo + 1) * P],
                                qch[:, c, :], start=(c == 0), stop=(c == 3),
                            )
                    for mo in range(2):
                        nc.vector.tensor_add(
                            QTs[fc][:, mo, :], ps[:, mo, :],
                            bq_sb[:, mo : mo + 1].to_broadcast((P, QCS)),
                        )

                def g_k():
                    ps = scp.tile([P, 2, QCS], f32, tag="sc", name=f"psk{fc}")
                    for mo in range(2):
                        for c in range(4):
                            nc.tensor.matmul(
                                ps[:, mo, :], wk_sb[:, c, mo * P : (mo + 1) * P],
                                kch[:, c, :], start=(c == 0), stop=(c == 3),
                            )
                    for mo in range(2):
                        nc.vector.tensor_add(
                            KTs[fc][:, mo, :], ps[:, mo, :],
                            bk_sb[:, mo : mo + 1].to_broadcast((P, QCS)),
                        )

                def g_v(k2):
                    if k2 == 0:
                        nc.vector.tensor_copy(Vgs[fc][:, :, :, DK], ones_r)
                    ps = scp.tile([P, 2, QCS], f32, tag="sc", name=f"psv{fc}{k2}")
                    for kl2 in range(2):
                        kl = k2 * 2 + kl2
                        for c in range(4):
                            nc.tensor.matmul(
                                ps[:, kl2, 0:DH],
                                vch[:, c, kl * P : (kl + 1) * P], wv_sb[:, c, :],
                                start=(c == 0), stop=(c == 3),
                            )
                    for kl2 in range(2):
                        kl = k2 * 2 + kl2
                        nc.vector.tensor_add(
                            Vgs[fc][:, kl, :, 0:DK],
                            ps[:, kl2, 0:DH].rearrange("p (h d) -> p h d", h=LH),
                            bv_r,
                        )

                return [g_q, g_k, lambda: g_v(0), lambda: g_v(1)]

            def oproj_groups(qc):
                qsl = slice(qc * QCS, (qc + 1) * QCS)

                def g_o(half):
                    ps = scp.tile([P, 2, QCS], f32, tag="sc", name=f"pso{qc}{half}")
                    for m2 in range(2):
                        mo = half * 2 + m2
                        msl = slice(mo * P, (mo + 1) * P)
                        for c in range(2):
                            nc.tensor.matmul(
                                ps[:, m2, :], wo_sb[:, c, msl], CTs[qc][:, c, :],
                                start=(c == 0), stop=(c == 1),
                            )
                    for m2 in range(2):
                        mo = half * 2 + m2
                        msl = slice(mo * P, (mo + 1) * P)
                        ost = opool.tile(
                            [P, QCS], f32, tag="ost", name=f"ost{qc}{mo}"
                        )
                        nc.vector.tensor_copy(ost[:], ps[:, m2, :])
                        nc.sync.dma_start(outT[msl, qsl], ost[:])

                return [lambda: g_o(0), lambda: g_o(1)]

            def attn(qc, fillers):
                # Software-pipelined: scores for kc are emitted BEFORE ctx for
                # kc-1 so the PE queue (in-order) never stalls on the ACT exp —
                # exp(kc) runs while the PE does ctx(kc-1)+scores(kc+1), and
                # filler groups slot in wherever ACT would otherwise gate.
                nkc = (qc + 1) * (QCS // KCS)

                def emit_scores(pr, kc):
                    fc, kk = kc // 4, kc % 4
                    ksl = slice(kk * KCS, (kk + 1) * KCS)
                    d = kc * KCS - qc * QCS
                    lo = max(d, 0)
                    sct = scp.tile([P, 2, QCS], f32, tag="sc")
                    nc.tensor.matmul(
                        sct[:, 0, lo:QCS], KTs[fc][0:DK, pr, ksl],
                        QTs[qc][0:DK, pr, lo:QCS], start=True, stop=True,
                    )
                    nc.tensor.matmul(
                        sct[:, 1, lo:QCS], KTs[fc][DK:P, pr, ksl],
                        QTs[qc][DK:P, pr, lo:QCS], start=True, stop=True,
                    )
                    if d >= 0:  # diagonal tile: causal window mask
                        nc.vector.tensor_add(
                            sct[:, 0, d : d + P], sct[:, 0, d : d + P], mt_sb[:]
                        )
                        nc.vector.tensor_add(
                            sct[:, 1, d : d + P], sct[:, 1, d : d + P], mt_sb[:]
                        )
                    et = epool.tile([P, 2, QCS], fr, tag="et")
                    nc.scalar.activation(et[:, :, lo:QCS], sct[:, :, lo:QCS], Exp)
                    return et, lo

                def emit_ctx(cA, cB, pr, kc, et, lo):
                    fc, kk = kc // 4, kc % 4
                    first, last = kc == 0, kc == nkc - 1
                    nc.tensor.matmul(
                        cA[0 : DK + 1, lo:QCS], Vgs[fc][:, kk, 2 * pr + 0, :],
                        et[:, 0, lo:QCS], start=first, stop=last,
                    )
                    nc.tensor.matmul(
                        cB[0 : DK + 1, lo:QCS], Vgs[fc][:, kk, 2 * pr + 1, :],
                        et[:, 1, lo:QCS], start=first, stop=last,
                    )

                # spread the filler groups evenly over the 2*nkc iterations,
                # skipping iteration 0 (let the attention pipeline prime)
                total_iters = 2 * nkc
                fill_at = {}
                for j, g in enumerate(fillers):
                    pos = 1 + (j * (total_iters - 1)) // max(len(fillers), 1)
                    fill_at.setdefault(pos, []).append(g)
                it = 0

                for pr in range(2):
                    cA = cxp.tile([P, QCS], f32, tag="cx0", name=f"cA{qc}{pr}")
                    cB = cxp.tile([P, QCS], f32, tag="cx1", name=f"cB{qc}{pr}")
                    et_p, lo_p = emit_scores(pr, 0)
                    for g in fill_at.get(it, ()):
                        g()
                    it += 1
                    for kc in range(1, nkc):
                        et, lo = emit_scores(pr, kc)
                        emit_ctx(cA, cB, pr, kc - 1, et_p, lo_p)
                        et_p, lo_p = et, lo
                        for g in fill_at.get(it, ()):
                            g()
                        it += 1
                    emit_ctx(cA, cB, pr, nkc - 1, et_p, lo_p)
                    # l rows sit on PSUM partition 64. Engines cannot shift
                    # partitions, so: DVE copy to SBUF (aligned), DMA the row
                    # down to partition 0, 2-ULP reciprocal there, then GPSIMD
                    # partition_broadcast across the 64 ctx partitions.
                    lr = lpool.tile([DK + 1, 2, QCS], f32, tag="lr")
                    nc.vector.tensor_copy(lr[DK : DK + 1, 0, :], cA[DK : DK + 1, :])
                    nc.vector.tensor_copy(lr[DK : DK + 1, 1, :], cB[DK : DK + 1, :])
                    l0 = lpool.tile([1, 2, QCS], f32, tag="l0")
                    nc.sync.dma_start(l0[:], lr[DK : DK + 1, :, :])
                    r0 = lpool.tile([1, 2, QCS], f32, tag="r0")
                    scr = lpool.tile([1, 2, QCS], f32, tag="scr")
                    nc.vector.reciprocal_approx_accurate(r0[:], l0[:], scr[:])
                    rbAs = cbpool.tile([DK, QCS], f32, tag="rbAs")
                    nc.gpsimd.partition_broadcast(rbAs[:], r0[0:1, 0, :], channels=DK)
                    rbBs = cbpool.tile([DK, QCS], f32, tag="rbBs")
                    nc.gpsimd.partition_broadcast(rbBs[:], r0[0:1, 1, :], channels=DK)
                    nc.vector.tensor_mul(CTs[qc][0:DK, pr, :], cA[0:DK, :], rbAs[:])
                    cbs = cbpool.tile([DK, QCS], fr, tag="cbs")
                    nc.vector.tensor_mul(cbs[:], cB[0:DK, :], rbBs[:])
                    # DMA moves it down to partitions 64..127 (DVE can't shift
                    # partitions; DMA can't read PSUM - hence the SBUF hop)
                    nc.sync.dma_start(CTs[qc][DK:P, pr, :], cbs[:])

            # Prologue: chunk 0 projection standalone (doubles as HAM warmup).
            ch0 = fetch_chunk(0)
            for g in proj_groups(0, *ch0):
                g()
            # attn(i) carries oproj(i-1) + proj(i+1) as PE filler groups.
            for i in range(NQC):
                fillers = []
                if i >= 1:
                    fillers += oproj_groups(i - 1)
                if i + 1 < NQC:
                    ch = fetch_chunk(i + 1)
                    fillers += proj_groups(i + 1, *ch)
                attn(i, fillers)
            for g in oproj_groups(NQC - 1):
                g()

    nc.compile()
    _CACHE["nc"] = nc
    return nc


def make_in_maps(q, k, v, w_q, b_q, w_k, b_k, w_v, b_v, w_o):
    """Host-side sharding: per-core input dict (all fp32, C-contiguous)."""
    f = np.float32
    q = np.asarray(q, f)
    k = np.asarray(k, f)
    v = np.asarray(v, f)
    w_q = np.asarray(w_q, f)
    w_k = np.asarray(w_k, f)
    w_v = np.asarray(w_v, f)
    w_o = np.asarray(w_o, f)
    b_q = np.asarray(b_q, f)
    b_k = np.asarray(b_k, f)
    b_v = np.asarray(b_v, f)

    scale = np.float32(1.0 / np.sqrt(DK))
    qTl = [np.ascontiguousarray(q[b].T) for b in range(B)]
    kTl = [np.ascontiguousarray(k[b].T) for b in range(B)]
    vTl = [np.ascontiguousarray(v[b].T) for b in range(B)]
    ii = np.arange(P)
    mtri = np.where(ii[:, None] > ii[None, :], f(NEG), f(0.0)).astype(f)

    per_g = []
    for g in range(HG):
        gsl = slice(g * DH, (g + 1) * DH)
        per_g.append(
            dict(
                wqT=np.ascontiguousarray(w_q[gsl, :].T),
                wkT=np.ascontiguousarray(w_k[gsl, :].T * scale),
                wvT=np.ascontiguousarray(w_v[gsl, :].T),
                woT=np.ascontiguousarray(w_o[:, gsl].T),
                bq=np.ascontiguousarray(b_q[gsl].reshape(2, P)),
                bk=np.ascontiguousarray((b_k[gsl] * scale).reshape(2, P)),
                bv=np.ascontiguousarray(np.broadcast_to(b_v[gsl], (P, DH))),
                mtri=mtri,
                onesd=np.ones((P, DK), f),
            )
        )

    in_maps = []
    for c in range(NCORES):
        b, g = c // HG, c % HG
        m = dict(qT=qTl[b], kT=kTl[b], vT=vTl[b], **per_g[g])
        in_maps.append(m)
    return in_maps


def gather(results, b_o):
    """Sum head-group partials per batch, un-transpose, add b_o."""
    b_o = np.asarray(b_o, np.float32)
    out = np.empty((B, S, D), np.float32)
    for b in range(B):
        acc = results[HG * b]["outT"] + results[HG * b + 1]["outT"]
        out[b] = acc.T + b_o
    return out


def kernel(q, k, v, mask, w_q, b_q, w_k, b_k, w_v, b_v, w_o, b_o, **run_kwargs):
    _import_concourse()
    from concourse.bass_utils import run_bass_kernel_spmd

    nc = build_nc()
    in_maps = make_in_maps(q, k, v, w_q, b_q, w_k, b_k, w_v, b_v, w_o)
    res = run_bass_kernel_spmd(nc, in_maps, core_ids=list(range(NCORES)), **run_kwargs)
    out = gather(res.results, b_o)
    kernel.last_result = res
    return out



# revision 11
# speedup vs baseline: 1.1930x; 1.1930x over previous
# Multi-head attention (B=4, S=2048, D=512, H=8) on 8 Trainium2 cores.
#
# Sharding: core c = (batch b=c//2, head-group g=c%2, 4 heads each). Every core
# runs the identical program (SPMD) on its own slice; partial w_o outputs of the
# two head-groups of a batch are summed on the host (+ b_o).
#
# Device dataflow keeps every activation transposed ([feature, token]) so no
# on-device transposes are needed:
#   QT = w_q_g @ qT + b  (PE, din on partitions)        [256, 2048]
#   KT = (w_k_g/8) @ kT + b/8                            [256, 2048]
#   V  = natural [keys, dout] via lhsT = vT chunks       [2048, 4, 65] (+ones col)
#   scores^T[k, q] = K Q^T  (lhsT = KT slice)            per (qc=512, kc=128)
#   E^T = exp(scores^T + causal window mask)             ACT, merged head-pair
#   ctx^T/l = V_aug^T E^T   (m=65: row 64 = l[q])        PSUM accumulate over kc
#   out^T_partial = w_oT_g ctx^T                         [512, 2048] -> DRAM
#
# All matmul operands are float32r (full-rate fp32 on the PE; producers round
# on write). Resident tensors are split into per-512-chunk tiles so phases
# overlap, and each q-chunk's output projection is emitted inside the next
# chunk's attention loop to keep the PE stream dense (HAM stays warm).
import os
import sys

import numpy as np

B, S, D, H = 4, 2048, 512, 8
DK = D // H          # 64
P = 128
NCORES = 8
HG = 2               # head groups (cores per batch)
DH = D // HG         # 256 out dims per core
LH = H // HG         # 4 local heads
QCS = 512            # q/key chunk size
NQC = S // QCS       # 4
KCS = 128            # key tile size (scores psum partition dim)
NEG = -1e9

# "bf16" (half-width operands: FWL weight loads, single-XBUS moving reads),
# "f32r" (replicated fp32, 1 cyc/row at free>=256) or "f32" (exact, 4 cyc/row)
MM_DT = os.environ.get("KERNEL_MM_DT", "bf16")

_CACHE = {}


def _import_concourse():
    for p in ("/opt/trn_rl_repo", "/root/.axon_site/_ro/trn_rl_repo"):
        if os.path.isdir(p) and p not in sys.path:
            sys.path.append(p)
    import concourse.bass as bass          # noqa: F401
    import concourse.mybir as mybir        # noqa: F401
    import concourse.tile as tile          # noqa: F401
    from concourse import bacc             # noqa: F401
    return bass, mybir, tile


def build_nc():
    """Build the (single, shared-by-all-cores) Bass program."""
    if "nc" in _CACHE:
        return _CACHE["nc"]
    bass, mybir, tile = _import_concourse()
    from concourse import bacc
    from contextlib import ExitStack

    f32 = mybir.dt.float32
    if MM_DT == "bf16":
        fr = mybir.dt.bfloat16
    elif MM_DT == "f32r":
        fr = mybir.dt.float32r
    else:
        fr = f32
    Exp = mybir.ActivationFunctionType.Exp

    nc = bacc.Bacc("TRN2", target_bir_lowering=False, debug=False)

    qT = nc.dram_tensor("qT", [D, S], fr, kind="ExternalInput").ap()
    kT = nc.dram_tensor("kT", [D, S], fr, kind="ExternalInput").ap()
    vT = nc.dram_tensor("vT", [D, S], fr, kind="ExternalInput").ap()
    wqT = nc.dram_tensor("wqT", [D, DH], fr, kind="ExternalInput").ap()
    wkT = nc.dram_tensor("wkT", [D, DH], fr, kind="ExternalInput").ap()
    wvT = nc.dram_tensor("wvT", [D, DH], fr, kind="ExternalInput").ap()
    woT = nc.dram_tensor("woT", [DH, D], fr, kind="ExternalInput").ap()
    bq = nc.dram_tensor("bq", [2, P], f32, kind="ExternalInput").ap()
    bk = nc.dram_tensor("bk", [2, P], f32, kind="ExternalInput").ap()
    bv = nc.dram_tensor("bv", [P, DH], f32, kind="ExternalInput").ap()
    mtri = nc.dram_tensor("mtri", [P, P], f32, kind="ExternalInput").ap()
    onesd = nc.dram_tensor("onesd", [P, DK], f32, kind="ExternalInput").ap()
    outT = nc.dram_tensor("outT", [D, S], f32, kind="ExternalOutput").ap()

    with tile.TileContext(nc) as tc, ExitStack() as ctx:
        wpool = ctx.enter_context(tc.tile_pool(name="weights", bufs=1))
        res = ctx.enter_context(tc.tile_pool(name="resident", bufs=1))
        opool = ctx.enter_context(tc.tile_pool(name="ost", bufs=4))

        wq_sb = wpool.tile([P, 4, DH], fr, tag="wq")
        nc.sync.dma_start(wq_sb[:], wqT.rearrange("(c p) m -> p c m", p=P))
        wk_sb = wpool.tile([P, 4, DH], fr, tag="wk")
        nc.sync.dma_start(wk_sb[:], wkT.rearrange("(c p) m -> p c m", p=P))
        wv_sb = wpool.tile([P, 4, DH], fr, tag="wv")
        nc.sync.dma_start(wv_sb[:], wvT.rearrange("(c p) m -> p c m", p=P))
        wo_sb = wpool.tile([P, 2, D], fr, tag="wo")
        nc.sync.dma_start(wo_sb[:], woT.rearrange("(c p) m -> p c m", p=P))
        bq_sb = wpool.tile([P, 2], f32, tag="bq")
        nc.sync.dma_start(bq_sb[:], bq.rearrange("c p -> p c"))
        bk_sb = wpool.tile([P, 2], f32, tag="bk")
        nc.sync.dma_start(bk_sb[:], bk.rearrange("c p -> p c"))
        bv_sb = wpool.tile([P, DH], f32, tag="bv")
        nc.sync.dma_start(bv_sb[:], bv[:])
        mt_sb = wpool.tile([P, P], f32, tag="mtri")
        nc.sync.dma_start(mt_sb[:], mtri[:])
        ones_sb = wpool.tile([P, DK], f32, tag="ones")
        nc.sync.dma_start(ones_sb[:], onesd[:])

        # per-512-chunk resident tiles -> fine-grained cross-phase deps
        QTs = [res.tile([P, 2, QCS], fr, tag=f"QT{i}", name=f"QT{i}") for i in range(NQC)]
        KTs = [res.tile([P, 2, QCS], fr, tag=f"KT{i}", name=f"KT{i}") for i in range(NQC)]
        Vgs = [
            res.tile([P, 4, LH, DK + 1], fr, tag=f"Vg{i}", name=f"Vg{i}")
            for i in range(NQC)
        ]
        CTs = [res.tile([P, 2, QCS], fr, tag=f"CT{i}", name=f"CT{i}") for i in range(NQC)]

        qT_r = qT.rearrange("(c p) f -> p c f", p=P)
        kT_r = kT.rearrange("(c p) f -> p c f", p=P)
        vT_r = vT.rearrange("(c p) f -> p c f", p=P)
        bv_r = bv_sb.rearrange("p (h d) -> p h d", h=LH)
        ones_r = ones_sb[:, 0 : 4 * LH].rearrange("p (a b) -> p a b", a=4)

        # ---- Unified pipeline ----
        # For i in 0..3: project 512-chunk i, then attention for q-chunk i
        # (causal: it only consumes chunks <= i), then the previous chunk's
        # output projection. Projections share the scores PSUM pool so the
        # whole kernel fits the 8 banks and the PE stream never breaks.
        with (
            tc.tile_pool(name="inq", bufs=2) as qpool,
            tc.tile_pool(name="ink", bufs=2) as kpool,
            tc.tile_pool(name="inv", bufs=2) as vpool,
            tc.tile_pool(name="et", bufs=6) as epool,
            tc.tile_pool(name="sc", bufs=2, space="PSUM") as scp,
            tc.tile_pool(name="cx", bufs=2, space="PSUM") as cxp,
            tc.tile_pool(name="ls", bufs=1) as lpool,
            tc.tile_pool(name="cbst", bufs=2) as cbpool,
        ):

            # Projections, output projections and attention all share one PE
            # instruction stream: proj/oproj matmul groups are interleaved as
            # "fillers" between attention iterations. A filler allocates from
            # the same "sc" PSUM rotation as the score tiles, so it only ever
            # waits on an exp already in flight (never on anything behind it
            # in the in-order PE queue). The PE therefore never has a
            # low-duty window and the HAM clock-gate stays at 2.4 GHz.
            def fetch_chunk(fc):
                sl = slice(fc * QCS, (fc + 1) * QCS)
                kch = kpool.tile([P, 4, QCS], fr, tag="kch", name=f"kch{fc}")
                nc.sync.dma_start(kch[:], kT_r[:, :, sl])
                qch = qpool.tile([P, 4, QCS], fr, tag="qch", name=f"qch{fc}")
                nc.sync.dma_start(qch[:], qT_r[:, :, sl])
                vch = vpool.tile([P, 4, QCS], fr, tag="vch", name=f"vch{fc}")
                nc.sync.dma_start(vch[:], vT_r[:, :, sl])
                return qch, kch, vch

            def proj_groups(fc, qch, kch, vch):
                def g_q():
                    ps = scp.tile([P, 2, QCS], f32, tag="sc", name=f"psq{fc}")
                    for mo in range(2):
                        for c in range(4):
                            nc.tensor.matmul(
                                ps[:, mo, :], wq_sb[:, c, mo * P : (mo + 1) * P],
                                qch[:, c, :], start=(c == 0), stop=(c == 3),
                            )
                    for mo in range(2):
                        nc.vector.tensor_add(
                            QTs[fc][:, mo, :], ps[:, mo, :],
                            bq_sb[:, mo : mo + 1].to_broadcast((P, QCS)),
                        )

                def g_k():
                    ps = scp.tile([P, 2, QCS], f32, tag="sc", name=f"psk{fc}")
                    for mo in range(2):
                        for c in range(4):
                            nc.tensor.matmul(
                                ps[:, mo, :], wk_sb[:, c, mo * P : (mo + 1) * P],
                                kch[:, c, :], start=(c == 0), stop=(c == 3),
                            )
                    for mo in range(2):
                        nc.vector.tensor_add(
                            KTs[fc][:, mo, :], ps[:, mo, :],
                            bk_sb[:, mo : mo + 1].to_broadcast((P, QCS)),
                        )

                def g_v(k2):
                    if k2 == 0:
                        nc.vector.tensor_copy(Vgs[fc][:, :, :, DK], ones_r)
                    ps = scp.tile([P, 2, QCS], f32, tag="sc", name=f"psv{fc}{k2}")
                    for kl2 in range(2):
                        kl = k2 * 2 + kl2
                        for c in range(4):
                            nc.tensor.matmul(
                                ps[:, kl2, 0:DH],
                                vch[:, c, kl * P : (kl + 1) * P], wv_sb[:, c, :],
                                start=(c == 0), stop=(c == 3),
                            )
                    for kl2 in range(2):
                        kl = k2 * 2 + kl2
                        nc.vector.tensor_add(
                            Vgs[fc][:, kl, :, 0:DK],
                            ps[:, kl2, 0:DH].rearrange("p (h d) -> p h d", h=LH),
                            bv_r,
                        )

                return [g_q, g_k, lambda: g_v(0), lambda: g_v(1)]

            def oproj_groups(qc):
                qsl = slice(qc * QCS, (qc + 1) * QCS)

                def g_o(half):
                    ps = scp.tile([P, 2, QCS], f32, tag="sc", name=f"pso{qc}{half}")
                    for m2 in range(2):
                        mo = half * 2 + m2
                        msl = slice(mo * P, (mo + 1) * P)
                        for c in range(2):
                            nc.tensor.matmul(
                                ps[:, m2, :], wo_sb[:, c, msl], CTs[qc][:, c, :],
                                start=(c == 0), stop=(c == 1),
                            )
                    for m2 in range(2):
                        mo = half * 2 + m2
                        msl = slice(mo * P, (mo + 1) * P)
                        ost = opool.tile(
                            [P, QCS], f32, tag="ost", name=f"ost{qc}{mo}"
                        )
                        nc.vector.tensor_copy(ost[:], ps[:, m2, :])
                        nc.sync.dma_start(outT[msl, qsl], ost[:])

                return [lambda: g_o(0), lambda: g_o(1)]

            def attn(qc, fillers):
                # Software-pipelined: scores for kc are emitted BEFORE ctx for
                # kc-1 so the PE queue (in-order) never stalls on the ACT exp —
                # exp(kc) runs while the PE does ctx(kc-1)+scores(kc+1), and
                # filler groups slot in wherever ACT would otherwise gate.
                nkc = (qc + 1) * (QCS // KCS)

                def emit_scores(pr, kc):
                    fc, kk = kc // 4, kc % 4
                    ksl = slice(kk * KCS, (kk + 1) * KCS)
                    d = kc * KCS - qc * QCS
                    lo = max(d, 0)
                    sct = scp.tile([P, 2, QCS], f32, tag="sc")
                    nc.tensor.matmul(
                        sct[:, 0, lo:QCS], KTs[fc][0:DK, pr, ksl],
                        QTs[qc][0:DK, pr, lo:QCS], start=True, stop=True,
                    )
                    nc.tensor.matmul(
                        sct[:, 1, lo:QCS], KTs[fc][DK:P, pr, ksl],
                        QTs[qc][DK:P, pr, lo:QCS], start=True, stop=True,
                    )
                    if d >= 0:  # diagonal tile: causal window mask
                        nc.vector.tensor_add(
                            sct[:, 0, d : d + P], sct[:, 0, d : d + P], mt_sb[:]
                        )
                        nc.vector.tensor_add(
                            sct[:, 1, d : d + P], sct[:, 1, d : d + P], mt_sb[:]
                        )
                    et = epool.tile([P, 2, QCS], fr, tag="et")
                    nc.scalar.activation(et[:, :, lo:QCS], sct[:, :, lo:QCS], Exp)
                    return et, lo

                def emit_ctx(cA, cB, pr, kc, et, lo):
                    fc, kk = kc // 4, kc % 4
                    first, last = kc == 0, kc == nkc - 1
                    nc.tensor.matmul(
                        cA[0 : DK + 1, lo:QCS], Vgs[fc][:, kk, 2 * pr + 0, :],
                        et[:, 0, lo:QCS], start=first, stop=last,
                    )
                    nc.tensor.matmul(
                        cB[0 : DK + 1, lo:QCS], Vgs[fc][:, kk, 2 * pr + 1, :],
                        et[:, 1, lo:QCS], start=first, stop=last,
                    )

                # spread the filler groups evenly over the 2*nkc iterations,
                # skipping iteration 0 (let the attention pipeline prime)
                total_iters = 2 * nkc
                fill_at = {}
                for j, g in enumerate(fillers):
                    pos = 1 + (j * (total_iters - 1)) // max(len(fillers), 1)
                    fill_at.setdefault(pos, []).append(g)
                it = 0

                for pr in range(2):
                    cA = cxp.tile([P, QCS], f32, tag="cx0", name=f"cA{qc}{pr}")
                    cB = cxp.tile([P, QCS], f32, tag="cx1", name=f"cB{qc}{pr}")
                    et_p, lo_p = emit_scores(pr, 0)
                    for g in fill_at.get(it, ()):
                        g()
                    it += 1
                    for kc in range(1, nkc):
                        et, lo = emit_scores(pr, kc)
                        emit_ctx(cA, cB, pr, kc - 1, et_p, lo_p)
                        et_p, lo_p = et, lo
                        for g in fill_at.get(it, ()):
                            g()
                        it += 1
                    emit_ctx(cA, cB, pr, nkc - 1, et_p, lo_p)
                    # l rows sit on PSUM partition 64. Engines cannot shift
                    # partitions, so: DVE copy to SBUF (aligned), DMA the row
                    # down to partition 0, 2-ULP reciprocal there, then GPSIMD
                    # partition_broadcast across the 64 ctx partitions.
                    lr = lpool.tile([DK + 1, 2, QCS], f32, tag="lr")
                    nc.vector.tensor_copy(lr[DK : DK + 1, 0, :], cA[DK : DK + 1, :])
                    nc.vector.tensor_copy(lr[DK : DK + 1, 1, :], cB[DK : DK + 1, :])
                    l0 = lpool.tile([1, 2, QCS], f32, tag="l0")
                    nc.sync.dma_start(l0[:], lr[DK : DK + 1, :, :])
                    r0 = lpool.tile([1, 2, QCS], f32, tag="r0")
                    nc.vector.reciprocal_approx_fast(r0[:], l0[:])
                    rbAs = cbpool.tile([DK, QCS], f32, tag="rbAs")
                    nc.gpsimd.partition_broadcast(rbAs[:], r0[0:1, 0, :], channels=DK)
                    rbBs = cbpool.tile([DK, QCS], f32, tag="rbBs")
                    nc.gpsimd.partition_broadcast(rbBs[:], r0[0:1, 1, :], channels=DK)
                    nc.vector.tensor_mul(CTs[qc][0:DK, pr, :], cA[0:DK, :], rbAs[:])
                    cbs = cbpool.tile([DK, QCS], fr, tag="cbs")
                    nc.vector.tensor_mul(cbs[:], cB[0:DK, :], rbBs[:])
                    # DMA moves it down to partitions 64..127 (DVE can't shift
                    # partitions; DMA can't read PSUM - hence the SBUF hop)
                    nc.sync.dma_start(CTs[qc][DK:P, pr, :], cbs[:])

            # Prologue: chunk 0 projection standalone (doubles as HAM warmup).
            ch0 = fetch_chunk(0)
            for g in proj_groups(0, *ch0):
                g()
            # attn(i) carries oproj(i-1) + proj(i+1) as PE filler groups.
            for i in range(NQC):
                fillers = []
                if i >= 1:
                    fillers += oproj_groups(i - 1)
                if i + 1 < NQC:
                    ch = fetch_chunk(i + 1)
                    fillers += proj_groups(i + 1, *ch)
                attn(i, fillers)
            for g in oproj_groups(NQC - 1):
                g()

    nc.compile()
    _CACHE["nc"] = nc
    return nc


def make_in_maps(q, k, v, w_q, b_q, w_k, b_k, w_v, b_v, w_o):
    """Host-side sharding: per-core input dict (all fp32, C-contiguous)."""
    f = np.float32
    q = np.asarray(q, f)
    k = np.asarray(k, f)
    v = np.asarray(v, f)
    w_q = np.asarray(w_q, f)
    w_k = np.asarray(w_k, f)
    w_v = np.asarray(w_v, f)
    w_o = np.asarray(w_o, f)
    b_q = np.asarray(b_q, f)
    b_k = np.asarray(b_k, f)
    b_v = np.asarray(b_v, f)

    if MM_DT == "bf16":
        import ml_dtypes

        mdt = ml_dtypes.bfloat16
    else:
        mdt = f

    scale = np.float32(1.0 / np.sqrt(DK))
    qTl = [np.ascontiguousarray(q[b].T.astype(mdt)) for b in range(B)]
    kTl = [np.ascontiguousarray(k[b].T.astype(mdt)) for b in range(B)]
    vTl = [np.ascontiguousarray(v[b].T.astype(mdt)) for b in range(B)]
    ii = np.arange(P)
    mtri = np.where(ii[:, None] > ii[None, :], f(NEG), f(0.0)).astype(f)

    per_g = []
    for g in range(HG):
        gsl = slice(g * DH, (g + 1) * DH)
        per_g.append(
            dict(
                wqT=np.ascontiguousarray(w_q[gsl, :].T.astype(mdt)),
                wkT=np.ascontiguousarray((w_k[gsl, :].T * scale).astype(mdt)),
                wvT=np.ascontiguousarray(w_v[gsl, :].T.astype(mdt)),
                woT=np.ascontiguousarray(w_o[:, gsl].T.astype(mdt)),
                bq=np.ascontiguousarray(b_q[gsl].reshape(2, P)),
                bk=np.ascontiguousarray((b_k[gsl] * scale).reshape(2, P)),
                bv=np.ascontiguousarray(np.broadcast_to(b_v[gsl], (P, DH))),
                mtri=mtri,
                onesd=np.ones((P, DK), f),
            )
        )

    in_maps = []
    for c in range(NCORES):
        b, g = c // HG, c % HG
        m = dict(qT=qTl[b], kT=kTl[b], vT=vTl[b], **per_g[g])
        in_maps.append(m)
    return in_maps


def gather(results, b_o):
    """Sum head-group partials per batch, un-transpose, add b_o."""
    b_o = np.asarray(b_o, np.float32)
    out = np.empty((B, S, D), np.float32)
    for b in range(B):
        acc = results[HG * b]["outT"] + results[HG * b + 1]["outT"]
        out[b] = acc.T + b_o
    return out


def kernel(q, k, v, mask, w_q, b_q, w_k, b_k, w_v, b_v, w_o, b_o, **run_kwargs):
    _import_concourse()
    from concourse.bass_utils import run_bass_kernel_spmd

    nc = build_nc()
    in_maps = make_in_maps(q, k, v, w_q, b_q, w_k, b_k, w_v, b_v, w_o)
    res = run_bass_kernel_spmd(nc, in_maps, core_ids=list(range(NCORES)), **run_kwargs)
    out = gather(res.results, b_o)
    kernel.last_result = res
    return out



# revision 13
# speedup vs baseline: 1.2488x; 1.0468x over previous
# Multi-head attention (B=4, S=2048, D=512, H=8) on 8 Trainium2 cores.
#
# Sharding: core c = (batch b=c//2, head-group g=c%2, 4 heads each). Every core
# runs the identical program (SPMD) on its own slice; partial w_o outputs of the
# two head-groups of a batch are summed on the host (+ b_o).
#
# Device dataflow keeps every activation transposed ([feature, token]) so no
# on-device transposes are needed:
#   QT = w_q_g @ qT + b  (PE, din on partitions)        [256, 2048]
#   KT = (w_k_g/8) @ kT + b/8                            [256, 2048]
#   V  = natural [keys, dout] via lhsT = vT chunks       [2048, 4, 65] (+ones col)
#   scores^T[k, q] = K Q^T  (lhsT = KT slice)            per (qc=512, kc=128)
#   E^T = exp(scores^T + causal window mask)             ACT, merged head-pair
#   ctx^T/l = V_aug^T E^T   (m=65: row 64 = l[q])        PSUM accumulate over kc
#   out^T_partial = w_oT_g ctx^T                         [512, 2048] -> DRAM
#
# All matmul operands are float32r (full-rate fp32 on the PE; producers round
# on write). Resident tensors are split into per-512-chunk tiles so phases
# overlap, and each q-chunk's output projection is emitted inside the next
# chunk's attention loop to keep the PE stream dense (HAM stays warm).
import os
import sys

import numpy as np

B, S, D, H = 4, 2048, 512, 8
DK = D // H          # 64
P = 128
NCORES = 8
HG = 2               # head groups (cores per batch)
DH = D // HG         # 256 out dims per core
LH = H // HG         # 4 local heads
QCS = 512            # q/key chunk size
NQC = S // QCS       # 4
KCS = 128            # key tile size (scores psum partition dim)
NEG = -1e9

# "bf16" (half-width operands: FWL weight loads, single-XBUS moving reads),
# "f32r" (replicated fp32, 1 cyc/row at free>=256) or "f32" (exact, 4 cyc/row)
MM_DT = os.environ.get("KERNEL_MM_DT", "bf16")

_CACHE = {}


def _import_concourse():
    for p in ("/opt/trn_rl_repo", "/root/.axon_site/_ro/trn_rl_repo"):
        if os.path.isdir(p) and p not in sys.path:
            sys.path.append(p)
    import concourse.bass as bass          # noqa: F401
    import concourse.mybir as mybir        # noqa: F401
    import concourse.tile as tile          # noqa: F401
    from concourse import bacc             # noqa: F401
    return bass, mybir, tile


def build_nc():
    """Build the (single, shared-by-all-cores) Bass program."""
    if "nc" in _CACHE:
        return _CACHE["nc"]
    bass, mybir, tile = _import_concourse()
    from concourse import bacc
    from contextlib import ExitStack

    f32 = mybir.dt.float32
    if MM_DT == "bf16":
        fr = mybir.dt.bfloat16
    elif MM_DT == "f32r":
        fr = mybir.dt.float32r
    else:
        fr = f32
    Exp = mybir.ActivationFunctionType.Exp

    nc = bacc.Bacc("TRN2", target_bir_lowering=False, debug=False)

    qT = nc.dram_tensor("qT", [D, S], fr, kind="ExternalInput").ap()
    kT = nc.dram_tensor("kT", [D, S], fr, kind="ExternalInput").ap()
    vT = nc.dram_tensor("vT", [D, S], fr, kind="ExternalInput").ap()
    wqT = nc.dram_tensor("wqT", [D, DH], fr, kind="ExternalInput").ap()
    wkT = nc.dram_tensor("wkT", [D, DH], fr, kind="ExternalInput").ap()
    wvT = nc.dram_tensor("wvT", [D, DH], fr, kind="ExternalInput").ap()
    woT = nc.dram_tensor("woT", [DH, D], fr, kind="ExternalInput").ap()
    bq = nc.dram_tensor("bq", [2, P], f32, kind="ExternalInput").ap()
    bk = nc.dram_tensor("bk", [2, P], f32, kind="ExternalInput").ap()
    bv = nc.dram_tensor("bv", [P, DH], f32, kind="ExternalInput").ap()
    mtri = nc.dram_tensor("mtri", [P, P], f32, kind="ExternalInput").ap()
    onesd = nc.dram_tensor("onesd", [P, DK], f32, kind="ExternalInput").ap()
    outT = nc.dram_tensor("outT", [D, S], f32, kind="ExternalOutput").ap()

    with tile.TileContext(nc) as tc, ExitStack() as ctx:
        wpool = ctx.enter_context(tc.tile_pool(name="weights", bufs=1))
        res = ctx.enter_context(tc.tile_pool(name="resident", bufs=1))
        opool = ctx.enter_context(tc.tile_pool(name="ost", bufs=4))

        wq_sb = wpool.tile([P, 4, DH], fr, tag="wq")
        nc.sync.dma_start(wq_sb[:], wqT.rearrange("(c p) m -> p c m", p=P))
        wk_sb = wpool.tile([P, 4, DH], fr, tag="wk")
        nc.sync.dma_start(wk_sb[:], wkT.rearrange("(c p) m -> p c m", p=P))
        wv_sb = wpool.tile([P, 4, DH], fr, tag="wv")
        nc.sync.dma_start(wv_sb[:], wvT.rearrange("(c p) m -> p c m", p=P))
        wo_sb = wpool.tile([P, 2, D], fr, tag="wo")
        nc.sync.dma_start(wo_sb[:], woT.rearrange("(c p) m -> p c m", p=P))
        bq_sb = wpool.tile([P, 2], f32, tag="bq")
        nc.sync.dma_start(bq_sb[:], bq.rearrange("c p -> p c"))
        bk_sb = wpool.tile([P, 2], f32, tag="bk")
        nc.sync.dma_start(bk_sb[:], bk.rearrange("c p -> p c"))
        bv_sb = wpool.tile([P, DH], f32, tag="bv")
        nc.sync.dma_start(bv_sb[:], bv[:])
        mt_sb = wpool.tile([P, P], f32, tag="mtri")
        nc.sync.dma_start(mt_sb[:], mtri[:])
        ones_sb = wpool.tile([P, DK], f32, tag="ones")
        nc.sync.dma_start(ones_sb[:], onesd[:])

        # per-512-chunk resident tiles -> fine-grained cross-phase deps
        QTs = [res.tile([P, 2, QCS], fr, tag=f"QT{i}", name=f"QT{i}") for i in range(NQC)]
        KTs = [res.tile([P, 2, QCS], fr, tag=f"KT{i}", name=f"KT{i}") for i in range(NQC)]
        Vgs = [
            res.tile([P, 4, LH, DK + 1], fr, tag=f"Vg{i}", name=f"Vg{i}")
            for i in range(NQC)
        ]
        CTs = [res.tile([P, 2, QCS], fr, tag=f"CT{i}", name=f"CT{i}") for i in range(NQC)]

        qT_r = qT.rearrange("(c p) f -> p c f", p=P)
        kT_r = kT.rearrange("(c p) f -> p c f", p=P)
        vT_r = vT.rearrange("(c p) f -> p c f", p=P)
        bv_r = bv_sb.rearrange("p (h d) -> p h d", h=LH)
        ones_r = ones_sb[:, 0 : 4 * LH].rearrange("p (a b) -> p a b", a=4)

        # ---- Unified pipeline ----
        # For i in 0..3: project 512-chunk i, then attention for q-chunk i
        # (causal: it only consumes chunks <= i), then the previous chunk's
        # output projection. Projections share the scores PSUM pool so the
        # whole kernel fits the 8 banks and the PE stream never breaks.
        with (
            tc.tile_pool(name="inq", bufs=2) as qpool,
            tc.tile_pool(name="ink", bufs=2) as kpool,
            tc.tile_pool(name="inv", bufs=2) as vpool,
            tc.tile_pool(name="et", bufs=6) as epool,
            tc.tile_pool(name="sc", bufs=2, space="PSUM") as scp,
            tc.tile_pool(name="cx", bufs=2, space="PSUM") as cxp,
            tc.tile_pool(name="ls", bufs=1) as lpool,
            tc.tile_pool(name="cbst", bufs=2) as cbpool,
        ):

            # Projections, output projections and attention all share one PE
            # instruction stream: proj/oproj matmul groups are interleaved as
            # "fillers" between attention iterations. A filler allocates from
            # the same "sc" PSUM rotation as the score tiles, so it only ever
            # waits on an exp already in flight (never on anything behind it
            # in the in-order PE queue). The PE therefore never has a
            # low-duty window and the HAM clock-gate stays at 2.4 GHz.
            def fetch_chunk(fc):
                sl = slice(fc * QCS, (fc + 1) * QCS)
                kch = kpool.tile([P, 4, QCS], fr, tag="kch", name=f"kch{fc}")
                nc.sync.dma_start(kch[:], kT_r[:, :, sl])
                qch = qpool.tile([P, 4, QCS], fr, tag="qch", name=f"qch{fc}")
                nc.sync.dma_start(qch[:], qT_r[:, :, sl])
                vch = vpool.tile([P, 4, QCS], fr, tag="vch", name=f"vch{fc}")
                nc.sync.dma_start(vch[:], vT_r[:, :, sl])
                return qch, kch, vch

            def proj_groups(fc, qch, kch, vch):
                def g_q():
                    ps = scp.tile([P, 2, QCS], f32, tag="sc", bufs=3, name=f"psq{fc}")
                    for mo in range(2):
                        for c in range(4):
                            nc.tensor.matmul(
                                ps[:, mo, :], wq_sb[:, c, mo * P : (mo + 1) * P],
                                qch[:, c, :], start=(c == 0), stop=(c == 3),
                            )
                    for mo in range(2):
                        nc.vector.tensor_add(
                            QTs[fc][:, mo, :], ps[:, mo, :],
                            bq_sb[:, mo : mo + 1].to_broadcast((P, QCS)),
                        )

                def g_k():
                    ps = scp.tile([P, 2, QCS], f32, tag="sc", bufs=3, name=f"psk{fc}")
                    for mo in range(2):
                        for c in range(4):
                            nc.tensor.matmul(
                                ps[:, mo, :], wk_sb[:, c, mo * P : (mo + 1) * P],
                                kch[:, c, :], start=(c == 0), stop=(c == 3),
                            )
                    for mo in range(2):
                        nc.vector.tensor_add(
                            KTs[fc][:, mo, :], ps[:, mo, :],
                            bk_sb[:, mo : mo + 1].to_broadcast((P, QCS)),
                        )

                def g_v(k2):
                    if k2 == 0:
                        nc.vector.tensor_copy(Vgs[fc][:, :, :, DK], ones_r)
                    ps = scp.tile([P, 2, QCS], f32, tag="sc", bufs=3, name=f"psv{fc}{k2}")
                    for kl2 in range(2):
                        kl = k2 * 2 + kl2
                        for c in range(4):
                            nc.tensor.matmul(
                                ps[:, kl2, 0:DH],
                                vch[:, c, kl * P : (kl + 1) * P], wv_sb[:, c, :],
                                start=(c == 0), stop=(c == 3),
                            )
                    for kl2 in range(2):
                        kl = k2 * 2 + kl2
                        nc.vector.tensor_add(
                            Vgs[fc][:, kl, :, 0:DK],
                            ps[:, kl2, 0:DH].rearrange("p (h d) -> p h d", h=LH),
                            bv_r,
                        )

                return [g_q, g_k, lambda: g_v(0), lambda: g_v(1)]

            def oproj_groups(qc):
                qsl = slice(qc * QCS, (qc + 1) * QCS)

                def g_o(half):
                    ps = scp.tile([P, 2, QCS], f32, tag="sc", bufs=3, name=f"pso{qc}{half}")
                    for m2 in range(2):
                        mo = half * 2 + m2
                        msl = slice(mo * P, (mo + 1) * P)
                        for c in range(2):
                            nc.tensor.matmul(
                                ps[:, m2, :], wo_sb[:, c, msl], CTs[qc][:, c, :],
                                start=(c == 0), stop=(c == 1),
                            )
                    for m2 in range(2):
                        mo = half * 2 + m2
                        msl = slice(mo * P, (mo + 1) * P)
                        ost = opool.tile(
                            [P, QCS], f32, tag="ost", name=f"ost{qc}{mo}"
                        )
                        nc.vector.tensor_copy(ost[:], ps[:, m2, :])
                        nc.sync.dma_start(outT[msl, qsl], ost[:])

                return [lambda: g_o(0), lambda: g_o(1)]

            def attn(qc, fillers):
                # Lookahead-2 software pipeline: the PE queue runs
                #   s0 s1 s2 [fillers] c0 s3 c1 s4 c2 ... c(n-1)
                # while ACT runs e0 e1 e2 ... back-to-back. ctx(k) waits on
                # exp(k), but the scores for k+2 are already AHEAD of it in
                # the queue, so the ACT stream (the critical resource) never
                # gaps. Score PSUM is triple-buffered; the ctx accumulators
                # are single-buffered and their pair-boundary drain (l-chain)
                # is bridged by the filler groups emitted just before c0.
                nkc = (qc + 1) * (QCS // KCS)

                def emit_scores(pr, kc):
                    fc, kk = kc // 4, kc % 4
                    ksl = slice(kk * KCS, (kk + 1) * KCS)
                    d = kc * KCS - qc * QCS
                    lo = max(d, 0)
                    sct = scp.tile([P, 2, QCS], f32, tag="sc", bufs=3)
                    nc.tensor.matmul(
                        sct[:, 0, lo:QCS], KTs[fc][0:DK, pr, ksl],
                        QTs[qc][0:DK, pr, lo:QCS], start=True, stop=True,
                    )
                    nc.tensor.matmul(
                        sct[:, 1, lo:QCS], KTs[fc][DK:P, pr, ksl],
                        QTs[qc][DK:P, pr, lo:QCS], start=True, stop=True,
                    )
                    if d >= 0:  # diagonal tile: causal window mask
                        nc.vector.tensor_add(
                            sct[:, 0, d : d + P], sct[:, 0, d : d + P], mt_sb[:]
                        )
                        nc.vector.tensor_add(
                            sct[:, 1, d : d + P], sct[:, 1, d : d + P], mt_sb[:]
                        )
                    return sct, lo

                def emit_exp(sct, lo):
                    et = epool.tile([P, 2, QCS], fr, tag="et")
                    nc.scalar.activation(et[:, :, lo:QCS], sct[:, :, lo:QCS], Exp)
                    return et

                def emit_ctx(cA, cB, pr, kc, et, lo):
                    fc, kk = kc // 4, kc % 4
                    first, last = kc == 0, kc == nkc - 1
                    nc.tensor.matmul(
                        cA[0 : DK + 1, lo:QCS], Vgs[fc][:, kk, 2 * pr + 0, :],
                        et[:, 0, lo:QCS], start=first, stop=last,
                    )
                    nc.tensor.matmul(
                        cB[0 : DK + 1, lo:QCS], Vgs[fc][:, kk, 2 * pr + 1, :],
                        et[:, 1, lo:QCS], start=first, stop=last,
                    )

                # split the fillers across the two pair boundaries; they
                # retire PE work while c0 waits on the previous pair's drain
                half = (len(fillers) + 1) // 2
                fill_pr = {0: fillers[:half], 1: fillers[half:]}

                for pr in range(2):
                    cA = cxp.tile([P, QCS], f32, tag="cx0", bufs=1, name=f"cA{qc}{pr}")
                    cB = cxp.tile([P, QCS], f32, tag="cx1", bufs=1, name=f"cB{qc}{pr}")
                    pipe = []  # (sct, lo, et)
                    s, lo = emit_scores(pr, 0)
                    pipe.append([s, lo, None])
                    if nkc > 1:
                        s, lo = emit_scores(pr, 1)
                        pipe.append([s, lo, None])
                    pipe[0][2] = emit_exp(pipe[0][0], pipe[0][1])
                    for kc in range(2, nkc):
                        s, lo = emit_scores(pr, kc)
                        pipe.append([s, lo, None])
                        pipe[kc - 1][2] = emit_exp(pipe[kc - 1][0], pipe[kc - 1][1])
                        if kc == 2:
                            for g in fill_pr[pr]:
                                g()
                        emit_ctx(cA, cB, pr, kc - 2, pipe[kc - 2][2], pipe[kc - 2][1])
                        pipe[kc - 2][0] = None
                    pipe[nkc - 1][2] = emit_exp(pipe[nkc - 1][0], pipe[nkc - 1][1])
                    if nkc == 1:
                        for g in fill_pr[pr]:
                            g()
                    emit_ctx(cA, cB, pr, nkc - 2, pipe[nkc - 2][2], pipe[nkc - 2][1])
                    emit_ctx(cA, cB, pr, nkc - 1, pipe[nkc - 1][2], pipe[nkc - 1][1])
                    # l rows sit on PSUM partition 64. Engines cannot shift
                    # partitions, so: DVE copy to SBUF (aligned), DMA the row
                    # down to partition 0, 2-ULP reciprocal there, then GPSIMD
                    # partition_broadcast across the 64 ctx partitions.
                    lr = lpool.tile([DK + 1, 2, QCS], f32, tag="lr")
                    nc.vector.tensor_copy(lr[DK : DK + 1, 0, :], cA[DK : DK + 1, :])
                    nc.vector.tensor_copy(lr[DK : DK + 1, 1, :], cB[DK : DK + 1, :])
                    l0 = lpool.tile([1, 2, QCS], f32, tag="l0")
                    nc.sync.dma_start(l0[:], lr[DK : DK + 1, :, :])
                    r0 = lpool.tile([1, 2, QCS], f32, tag="r0")
                    nc.vector.reciprocal_approx_fast(r0[:], l0[:])
                    rbAs = cbpool.tile([DK, QCS], f32, tag="rbAs")
                    nc.gpsimd.partition_broadcast(rbAs[:], r0[0:1, 0, :], channels=DK)
                    rbBs = cbpool.tile([DK, QCS], f32, tag="rbBs")
                    nc.gpsimd.partition_broadcast(rbBs[:], r0[0:1, 1, :], channels=DK)
                    nc.vector.tensor_mul(CTs[qc][0:DK, pr, :], cA[0:DK, :], rbAs[:])
                    cbs = cbpool.tile([DK, QCS], fr, tag="cbs")
                    nc.vector.tensor_mul(cbs[:], cB[0:DK, :], rbBs[:])
                    # DMA moves it down to partitions 64..127 (DVE can't shift
                    # partitions; DMA can't read PSUM - hence the SBUF hop)
                    nc.sync.dma_start(CTs[qc][DK:P, pr, :], cbs[:])

            # Prologue: chunk 0 projection standalone (doubles as HAM warmup).
            ch0 = fetch_chunk(0)
            for g in proj_groups(0, *ch0):
                g()
            # attn(i) carries oproj(i-1) + proj(i+1) as PE filler groups.
            for i in range(NQC):
                fillers = []
                if i >= 1:
                    fillers += oproj_groups(i - 1)
                if i + 1 < NQC:
                    ch = fetch_chunk(i + 1)
                    fillers += proj_groups(i + 1, *ch)
                attn(i, fillers)
            for g in oproj_groups(NQC - 1):
                g()

    nc.compile()
    _CACHE["nc"] = nc
    return nc


def make_in_maps(q, k, v, w_q, b_q, w_k, b_k, w_v, b_v, w_o):
    """Host-side sharding: per-core input dict (all fp32, C-contiguous)."""
    f = np.float32
    q = np.asarray(q, f)
    k = np.asarray(k, f)
    v = np.asarray(v, f)
    w_q = np.asarray(w_q, f)
    w_k = np.asarray(w_k, f)
    w_v = np.asarray(w_v, f)
    w_o = np.asarray(w_o, f)
    b_q = np.asarray(b_q, f)
    b_k = np.asarray(b_k, f)
    b_v = np.asarray(b_v, f)

    if MM_DT == "bf16":
        import ml_dtypes

        mdt = ml_dtypes.bfloat16
    else:
        mdt = f

    scale = np.float32(1.0 / np.sqrt(DK))
    qTl = [np.ascontiguousarray(q[b].T.astype(mdt)) for b in range(B)]
    kTl = [np.ascontiguousarray(k[b].T.astype(mdt)) for b in range(B)]
    vTl = [np.ascontiguousarray(v[b].T.astype(mdt)) for b in range(B)]
    ii = np.arange(P)
    mtri = np.where(ii[:, None] > ii[None, :], f(NEG), f(0.0)).astype(f)

    per_g = []
    for g in range(HG):
        gsl = slice(g * DH, (g + 1) * DH)
        per_g.append(
            dict(
                wqT=np.ascontiguousarray(w_q[gsl, :].T.astype(mdt)),
                wkT=np.ascontiguousarray((w_k[gsl, :].T * scale).astype(mdt)),
                wvT=np.ascontiguousarray(w_v[gsl, :].T.astype(mdt)),
                woT=np.ascontiguousarray(w_o[:, gsl].T.astype(mdt)),
                bq=np.ascontiguousarray(b_q[gsl].reshape(2, P)),
                bk=np.ascontiguousarray((b_k[gsl] * scale).reshape(2, P)),
                bv=np.ascontiguousarray(np.broadcast_to(b_v[gsl], (P, DH))),
                mtri=mtri,
                onesd=np.ones((P, DK), f),
            )
        )

    in_maps = []
    for c in range(NCORES):
        b, g = c // HG, c % HG
        m = dict(qT=qTl[b], kT=kTl[b], vT=vTl[b], **per_g[g])
        in_maps.append(m)
    return in_maps


def gather(results, b_o):
    """Sum head-group partials per batch, un-transpose, add b_o."""
    b_o = np.asarray(b_o, np.float32)
    out = np.empty((B, S, D), np.float32)
    for b in range(B):
        acc = results[HG * b]["outT"] + results[HG * b + 1]["outT"]
        out[b] = acc.T + b_o
    return out


def kernel(q, k, v, mask, w_q, b_q, w_k, b_k, w_v, b_v, w_o, b_o, **run_kwargs):
    _import_concourse()
    from concourse.bass_utils import run_bass_kernel_spmd

    nc = build_nc()
    in_maps = make_in_maps(q, k, v, w_q, b_q, w_k, b_k, w_v, b_v, w_o)
    res = run_bass_kernel_spmd(nc, in_maps, core_ids=list(range(NCORES)), **run_kwargs)
    out = gather(res.results, b_o)
    kernel.last_result = res
    return out



# revision 18
# speedup vs baseline: 1.2815x; 1.0261x over previous
# Multi-head attention (B=4, S=2048, D=512, H=8) on 8 Trainium2 cores.
#
# Sharding: core c = (batch b=c//2, head-group g=c%2, 4 heads each). Every core
# runs the identical program (SPMD) on its own slice; partial w_o outputs of the
# two head-groups of a batch are summed on the host (+ b_o).
#
# Device dataflow keeps every activation transposed ([feature, token]) so no
# on-device transposes are needed:
#   QT = w_q_g @ qT + b  (PE, din on partitions)        [256, 2048]
#   KT = (w_k_g/8) @ kT + b/8                            [256, 2048]
#   V  = natural [keys, dout] via lhsT = vT chunks       [2048, 4, 65] (+ones col)
#   scores^T[k, q] = K Q^T  (lhsT = KT slice)            per (qc=512, kc=128)
#   E^T = exp(scores^T + causal window mask)             ACT, merged head-pair
#   ctx^T/l = V_aug^T E^T   (m=65: row 64 = l[q])        PSUM accumulate over kc
#   out^T_partial = w_oT_g ctx^T                         [512, 2048] -> DRAM
#
# All matmul operands are float32r (full-rate fp32 on the PE; producers round
# on write). Resident tensors are split into per-512-chunk tiles so phases
# overlap, and each q-chunk's output projection is emitted inside the next
# chunk's attention loop to keep the PE stream dense (HAM stays warm).
import os
import sys

import numpy as np

B, S, D, H = 4, 2048, 512, 8
DK = D // H          # 64
P = 128
NCORES = 8
HG = 2               # head groups (cores per batch)
DH = D // HG         # 256 out dims per core
LH = H // HG         # 4 local heads
QCS = 512            # q/key chunk size
NQC = S // QCS       # 4
KCS = 128            # key tile size (scores psum partition dim)
NEG = -1e9

# "bf16" (half-width operands: FWL weight loads, single-XBUS moving reads),
# "f32r" (replicated fp32, 1 cyc/row at free>=256) or "f32" (exact, 4 cyc/row)
MM_DT = os.environ.get("KERNEL_MM_DT", "bf16")

_CACHE = {}


def _import_concourse():
    for p in ("/opt/trn_rl_repo", "/root/.axon_site/_ro/trn_rl_repo"):
        if os.path.isdir(p) and p not in sys.path:
            sys.path.append(p)
    import concourse.bass as bass          # noqa: F401
    import concourse.mybir as mybir        # noqa: F401
    import concourse.tile as tile          # noqa: F401
    from concourse import bacc             # noqa: F401
    return bass, mybir, tile


def build_nc():
    """Build the (single, shared-by-all-cores) Bass program."""
    if "nc" in _CACHE:
        return _CACHE["nc"]
    bass, mybir, tile = _import_concourse()
    from concourse import bacc
    from contextlib import ExitStack

    f32 = mybir.dt.float32
    if MM_DT == "bf16":
        fr = mybir.dt.bfloat16
    elif MM_DT == "f32r":
        fr = mybir.dt.float32r
    else:
        fr = f32
    Exp = mybir.ActivationFunctionType.Exp

    nc = bacc.Bacc("TRN2", target_bir_lowering=False, debug=False)

    qT = nc.dram_tensor("qT", [D, S], fr, kind="ExternalInput").ap()
    kT = nc.dram_tensor("kT", [D, S], fr, kind="ExternalInput").ap()
    vT = nc.dram_tensor("vT", [D, S], fr, kind="ExternalInput").ap()
    wqT = nc.dram_tensor("wqT", [D, DH], fr, kind="ExternalInput").ap()
    wkT = nc.dram_tensor("wkT", [D, DH], fr, kind="ExternalInput").ap()
    wvT = nc.dram_tensor("wvT", [D, DH], fr, kind="ExternalInput").ap()
    woT = nc.dram_tensor("woT", [DH, D], fr, kind="ExternalInput").ap()
    bq = nc.dram_tensor("bq", [2, P], f32, kind="ExternalInput").ap()
    bk = nc.dram_tensor("bk", [2, P], f32, kind="ExternalInput").ap()
    bv = nc.dram_tensor("bv", [P, DH], f32, kind="ExternalInput").ap()
    mtri = nc.dram_tensor("mtri", [P, P], f32, kind="ExternalInput").ap()
    onesd = nc.dram_tensor("onesd", [P, DK], f32, kind="ExternalInput").ap()
    outT = nc.dram_tensor("outT", [D, S], fr, kind="ExternalOutput").ap()

    with tile.TileContext(nc) as tc, ExitStack() as ctx:
        wpool = ctx.enter_context(tc.tile_pool(name="weights", bufs=1))
        res = ctx.enter_context(tc.tile_pool(name="resident", bufs=1))
        opool = ctx.enter_context(tc.tile_pool(name="ost", bufs=4))

        # Per-128-column-slice DMAs so the first projection matmul can start
        # after ~1/12 of the input bytes land (subtile deps). Emission order
        # front-loads what the prologue Q projection needs; wo lands last.
        wq_r = wqT.rearrange("(c p) m -> p c m", p=P)
        wk_r = wkT.rearrange("(c p) m -> p c m", p=P)
        wv_r = wvT.rearrange("(c p) m -> p c m", p=P)
        wq_sb = wpool.tile([P, 4, DH], fr, tag="wq")
        for c in range(4):
            nc.sync.dma_start(wq_sb[:, c, :], wq_r[:, c, :])
        bq_sb = wpool.tile([P, 2], f32, tag="bq")
        nc.sync.dma_start(bq_sb[:], bq.rearrange("c p -> p c"))
        wk_sb = wpool.tile([P, 4, DH], fr, tag="wk")
        for c in range(4):
            nc.sync.dma_start(wk_sb[:, c, :], wk_r[:, c, :])
        bk_sb = wpool.tile([P, 2], f32, tag="bk")
        nc.sync.dma_start(bk_sb[:], bk.rearrange("c p -> p c"))
        wv_sb = wpool.tile([P, 4, DH], fr, tag="wv")
        for c in range(4):
            nc.sync.dma_start(wv_sb[:, c, :], wv_r[:, c, :])
        bv_sb = wpool.tile([P, DH], f32, tag="bv")
        nc.sync.dma_start(bv_sb[:], bv[:])
        mt_sb = wpool.tile([P, P], f32, tag="mtri")
        nc.sync.dma_start(mt_sb[:], mtri[:])
        ones_sb = wpool.tile([P, DK], f32, tag="ones")
        nc.sync.dma_start(ones_sb[:], onesd[:])
        wo_sb = wpool.tile([P, 2, D], fr, tag="wo")
        nc.sync.dma_start(wo_sb[:], woT.rearrange("(c p) m -> p c m", p=P))

        # per-512-chunk resident tiles -> fine-grained cross-phase deps
        QTs = [res.tile([P, 2, QCS], fr, tag=f"QT{i}", name=f"QT{i}") for i in range(NQC)]
        KTs = [res.tile([P, 2, QCS], fr, tag=f"KT{i}", name=f"KT{i}") for i in range(NQC)]
        Vgs = [
            res.tile([P, 4, LH, DK + 1], fr, tag=f"Vg{i}", name=f"Vg{i}")
            for i in range(NQC)
        ]
        CTs = [res.tile([P, 2, QCS], fr, tag=f"CT{i}", name=f"CT{i}") for i in range(NQC)]

        qT_r = qT.rearrange("(c p) f -> p c f", p=P)
        kT_r = kT.rearrange("(c p) f -> p c f", p=P)
        vT_r = vT.rearrange("(c p) f -> p c f", p=P)
        bv_r = bv_sb.rearrange("p (h d) -> p h d", h=LH)
        ones_r = ones_sb[:, 0 : 4 * LH].rearrange("p (a b) -> p a b", a=4)

        # ---- Unified pipeline ----
        # For i in 0..3: project 512-chunk i, then attention for q-chunk i
        # (causal: it only consumes chunks <= i), then the previous chunk's
        # output projection. Projections share the scores PSUM pool so the
        # whole kernel fits the 8 banks and the PE stream never breaks.
        with (
            tc.tile_pool(name="inq", bufs=2) as qpool,
            tc.tile_pool(name="ink", bufs=2) as kpool,
            tc.tile_pool(name="inv", bufs=2) as vpool,
            tc.tile_pool(name="et", bufs=8) as epool,
            tc.tile_pool(name="sc", bufs=2, space="PSUM") as scp,
            tc.tile_pool(name="cx", bufs=2, space="PSUM") as cxp,
            tc.tile_pool(name="ls", bufs=1) as lpool,
            tc.tile_pool(name="cbst", bufs=2) as cbpool,
        ):

            # Projections, output projections and attention all share one PE
            # instruction stream: proj/oproj matmul groups are interleaved as
            # "fillers" between attention iterations. A filler allocates from
            # the same "sc" PSUM rotation as the score tiles, so it only ever
            # waits on an exp already in flight (never on anything behind it
            # in the in-order PE queue). The PE therefore never has a
            # low-duty window and the HAM clock-gate stays at 2.4 GHz.
            def fetch_chunk(fc):
                sl = slice(fc * QCS, (fc + 1) * QCS)
                qch = qpool.tile([P, 4, QCS], fr, tag="qch", name=f"qch{fc}")
                for c in range(4):
                    nc.sync.dma_start(qch[:, c, :], qT_r[:, c, sl])
                kch = kpool.tile([P, 4, QCS], fr, tag="kch", name=f"kch{fc}")
                for c in range(4):
                    nc.sync.dma_start(kch[:, c, :], kT_r[:, c, sl])
                vch = vpool.tile([P, 4, QCS], fr, tag="vch", name=f"vch{fc}")
                for c in range(4):
                    nc.sync.dma_start(vch[:, c, :], vT_r[:, c, sl])
                return qch, kch, vch

            def proj_groups(fc, qch, kch, vch):
                def g_q():
                    ps = scp.tile([P, 2, QCS], f32, tag="sc", bufs=3, name=f"psq{fc}")
                    for mo in range(2):
                        for c in range(4):
                            nc.tensor.matmul(
                                ps[:, mo, :], wq_sb[:, c, mo * P : (mo + 1) * P],
                                qch[:, c, :], start=(c == 0), stop=(c == 3),
                            )
                    for mo in range(2):
                        nc.vector.tensor_add(
                            QTs[fc][:, mo, :], ps[:, mo, :],
                            bq_sb[:, mo : mo + 1].to_broadcast((P, QCS)),
                        )

                def g_k():
                    ps = scp.tile([P, 2, QCS], f32, tag="sc", bufs=3, name=f"psk{fc}")
                    for mo in range(2):
                        for c in range(4):
                            nc.tensor.matmul(
                                ps[:, mo, :], wk_sb[:, c, mo * P : (mo + 1) * P],
                                kch[:, c, :], start=(c == 0), stop=(c == 3),
                            )
                    for mo in range(2):
                        nc.vector.tensor_add(
                            KTs[fc][:, mo, :], ps[:, mo, :],
                            bk_sb[:, mo : mo + 1].to_broadcast((P, QCS)),
                        )

                def g_v(k2):
                    if k2 == 0:
                        nc.vector.tensor_copy(Vgs[fc][:, :, :, DK], ones_r)
                    ps = scp.tile([P, 2, QCS], f32, tag="sc", bufs=3, name=f"psv{fc}{k2}")
                    for kl2 in range(2):
                        kl = k2 * 2 + kl2
                        for c in range(4):
                            nc.tensor.matmul(
                                ps[:, kl2, 0:DH],
                                vch[:, c, kl * P : (kl + 1) * P], wv_sb[:, c, :],
                                start=(c == 0), stop=(c == 3),
                            )
                    for kl2 in range(2):
                        kl = k2 * 2 + kl2
                        nc.vector.tensor_add(
                            Vgs[fc][:, kl, :, 0:DK],
                            ps[:, kl2, 0:DH].rearrange("p (h d) -> p h d", h=LH),
                            bv_r,
                        )

                return [g_q, g_k, lambda: g_v(0), lambda: g_v(1)]

            def oproj_groups(qc):
                qsl = slice(qc * QCS, (qc + 1) * QCS)

                def g_o(half):
                    ps = scp.tile([P, 2, QCS], f32, tag="sc", bufs=3, name=f"pso{qc}{half}")
                    for m2 in range(2):
                        mo = half * 2 + m2
                        msl = slice(mo * P, (mo + 1) * P)
                        for c in range(2):
                            nc.tensor.matmul(
                                ps[:, m2, :], wo_sb[:, c, msl], CTs[qc][:, c, :],
                                start=(c == 0), stop=(c == 1),
                            )
                    for m2 in range(2):
                        mo = half * 2 + m2
                        msl = slice(mo * P, (mo + 1) * P)
                        ost = opool.tile(
                            [P, QCS], fr, tag="ost", name=f"ost{qc}{mo}"
                        )
                        nc.vector.tensor_copy(ost[:], ps[:, m2, :])
                        nc.sync.dma_start(outT[msl, qsl], ost[:])

                return [lambda: g_o(0), lambda: g_o(1)]

            def attn(qc, fillers):
                # Lookahead-2 software pipeline: the PE queue runs
                #   s0 s1 s2 [fillers] c0 s3 c1 s4 c2 ... c(n-1)
                # while ACT runs e0 e1 e2 ... back-to-back. ctx(k) waits on
                # exp(k), but the scores for k+2 are already AHEAD of it in
                # the queue, so the ACT stream (the critical resource) never
                # gaps. Score PSUM is triple-buffered; the ctx accumulators
                # are single-buffered and their pair-boundary drain (l-chain)
                # is bridged by the filler groups emitted just before c0.
                nkc = (qc + 1) * (QCS // KCS)

                def emit_scores(pr, kc):
                    fc, kk = kc // 4, kc % 4
                    ksl = slice(kk * KCS, (kk + 1) * KCS)
                    d = kc * KCS - qc * QCS
                    lo = max(d, 0)
                    sct = scp.tile([P, 2, QCS], f32, tag="sc", bufs=3)
                    nc.tensor.matmul(
                        sct[:, 0, lo:QCS], KTs[fc][0:DK, pr, ksl],
                        QTs[qc][0:DK, pr, lo:QCS], start=True, stop=True,
                    )
                    nc.tensor.matmul(
                        sct[:, 1, lo:QCS], KTs[fc][DK:P, pr, ksl],
                        QTs[qc][DK:P, pr, lo:QCS], start=True, stop=True,
                    )
                    if d >= 0:  # diagonal tile: causal window mask
                        nc.vector.tensor_add(
                            sct[:, 0, d : d + P], sct[:, 0, d : d + P], mt_sb[:]
                        )
                        nc.vector.tensor_add(
                            sct[:, 1, d : d + P], sct[:, 1, d : d + P], mt_sb[:]
                        )
                    return sct, lo

                def emit_exp(sct, lo):
                    et = epool.tile([P, 2, QCS], fr, tag="et")
                    nc.scalar.activation(et[:, :, lo:QCS], sct[:, :, lo:QCS], Exp)
                    return et

                def emit_ctx(cA, cB, pr, kc, et, lo):
                    fc, kk = kc // 4, kc % 4
                    first, last = kc == 0, kc == nkc - 1
                    nc.tensor.matmul(
                        cA[0 : DK + 1, lo:QCS], Vgs[fc][:, kk, 2 * pr + 0, :],
                        et[:, 0, lo:QCS], start=first, stop=last,
                    )
                    nc.tensor.matmul(
                        cB[0 : DK + 1, lo:QCS], Vgs[fc][:, kk, 2 * pr + 1, :],
                        et[:, 1, lo:QCS], start=first, stop=last,
                    )

                # Fillers pop one per iteration slot (kc>=2) so they never
                # starve the ACT stream; ~5 exps of prime cover the previous
                # pair's l-chain drain before c0 blocks the in-order queue.
                PRIME = min(5, nkc - 1)
                fq = list(fillers)

                for pr in range(2):
                    cA = cxp.tile([P, QCS], f32, tag="cx0", bufs=1, name=f"cA{qc}{pr}")
                    cB = cxp.tile([P, QCS], f32, tag="cx1", bufs=1, name=f"cB{qc}{pr}")
                    pipe = []  # (sct, lo, et)
                    cc = 0
                    for kc in range(nkc):
                        s, lo = emit_scores(pr, kc)
                        pipe.append([s, lo, emit_exp(s, lo)])
                        if fq and kc >= 2:
                            fq.pop(0)()
                        if kc >= PRIME:
                            emit_ctx(cA, cB, pr, cc, pipe[cc][2], pipe[cc][1])
                            cc += 1
                            if cc <= kc - 2:
                                emit_ctx(cA, cB, pr, cc, pipe[cc][2], pipe[cc][1])
                                cc += 1
                    while cc < nkc:
                        emit_ctx(cA, cB, pr, cc, pipe[cc][2], pipe[cc][1])
                        cc += 1
                    # l rows sit on PSUM partition 64. Per head: DVE copy to
                    # SBUF, DMA the row to partition 0, reciprocal, GPSIMD
                    # partition_broadcast, multiply — split per head so each
                    # accumulator frees as early as possible.
                    lr = lpool.tile([DK + 1, 2, QCS], f32, tag="lr")
                    l0 = lpool.tile([1, 2, QCS], f32, tag="l0")
                    r0 = lpool.tile([1, 2, QCS], f32, tag="r0")
                    nc.vector.tensor_copy(lr[DK : DK + 1, 0, :], cA[DK : DK + 1, :])
                    nc.sync.dma_start(l0[:, 0, :], lr[DK : DK + 1, 0, :])
                    nc.vector.tensor_copy(lr[DK : DK + 1, 1, :], cB[DK : DK + 1, :])
                    nc.sync.dma_start(l0[:, 1, :], lr[DK : DK + 1, 1, :])
                    nc.vector.reciprocal_approx_fast(r0[:, 0, :], l0[:, 0, :])
                    rbAs = cbpool.tile([DK, QCS], f32, tag="rbAs")
                    nc.gpsimd.partition_broadcast(rbAs[:], r0[0:1, 0, :], channels=DK)
                    nc.vector.reciprocal_approx_fast(r0[:, 1, :], l0[:, 1, :])
                    rbBs = cbpool.tile([DK, QCS], f32, tag="rbBs")
                    nc.gpsimd.partition_broadcast(rbBs[:], r0[0:1, 1, :], channels=DK)
                    nc.vector.tensor_mul(CTs[qc][0:DK, pr, :], cA[0:DK, :], rbAs[:])
                    cbs = cbpool.tile([DK, QCS], fr, tag="cbs")
                    nc.vector.tensor_mul(cbs[:], cB[0:DK, :], rbBs[:])
                    # DMA moves it down to partitions 64..127 (DVE can't shift
                    # partitions; DMA can't read PSUM - hence the SBUF hop)
                    nc.sync.dma_start(CTs[qc][DK:P, pr, :], cbs[:])

            # Prologue: chunk 0 projection standalone (doubles as HAM warmup).
            ch0 = fetch_chunk(0)
            for g in proj_groups(0, *ch0):
                g()
            # attn(i) carries oproj(i-1) + proj(i+1) as PE filler groups.
            # Chunk 3's K/V groups ride inside attn(3) itself (they are only
            # needed from kc=12) so attn(3)'s pair boundaries have cover too.
            gq3 = gk3 = gv30 = gv31 = None
            for i in range(NQC):
                fillers = []
                if i >= 1:
                    fillers += oproj_groups(i - 1)
                if i + 1 < NQC:
                    ch = fetch_chunk(i + 1)
                    gs = proj_groups(i + 1, *ch)
                    if i + 1 == 3:
                        gq3, gk3, gv30, gv31 = gs
                        fillers += [gq3]
                    else:
                        fillers += gs
                elif i == NQC - 1:
                    fillers += [gk3, gv30, gv31]
                attn(i, fillers)
            for g in oproj_groups(NQC - 1):
                g()

    nc.compile()
    _CACHE["nc"] = nc
    return nc


def make_in_maps(q, k, v, w_q, b_q, w_k, b_k, w_v, b_v, w_o):
    """Host-side sharding: per-core input dict (all fp32, C-contiguous)."""
    f = np.float32
    q = np.asarray(q, f)
    k = np.asarray(k, f)
    v = np.asarray(v, f)
    w_q = np.asarray(w_q, f)
    w_k = np.asarray(w_k, f)
    w_v = np.asarray(w_v, f)
    w_o = np.asarray(w_o, f)
    b_q = np.asarray(b_q, f)
    b_k = np.asarray(b_k, f)
    b_v = np.asarray(b_v, f)

    if MM_DT == "bf16":
        import ml_dtypes

        mdt = ml_dtypes.bfloat16
    else:
        mdt = f

    scale = np.float32(1.0 / np.sqrt(DK))
    qTl = [np.ascontiguousarray(q[b].T.astype(mdt)) for b in range(B)]
    kTl = [np.ascontiguousarray(k[b].T.astype(mdt)) for b in range(B)]
    vTl = [np.ascontiguousarray(v[b].T.astype(mdt)) for b in range(B)]
    ii = np.arange(P)
    mtri = np.where(ii[:, None] > ii[None, :], f(NEG), f(0.0)).astype(f)

    per_g = []
    for g in range(HG):
        gsl = slice(g * DH, (g + 1) * DH)
        per_g.append(
            dict(
                wqT=np.ascontiguousarray(w_q[gsl, :].T.astype(mdt)),
                wkT=np.ascontiguousarray((w_k[gsl, :].T * scale).astype(mdt)),
                wvT=np.ascontiguousarray(w_v[gsl, :].T.astype(mdt)),
                woT=np.ascontiguousarray(w_o[:, gsl].T.astype(mdt)),
                bq=np.ascontiguousarray(b_q[gsl].reshape(2, P)),
                bk=np.ascontiguousarray((b_k[gsl] * scale).reshape(2, P)),
                bv=np.ascontiguousarray(np.broadcast_to(b_v[gsl], (P, DH))),
                mtri=mtri,
                onesd=np.ones((P, DK), f),
            )
        )

    in_maps = []
    for c in range(NCORES):
        b, g = c // HG, c % HG
        m = dict(qT=qTl[b], kT=kTl[b], vT=vTl[b], **per_g[g])
        in_maps.append(m)
    return in_maps


def gather(results, b_o):
    """Sum head-group partials per batch, un-transpose, add b_o."""
    b_o = np.asarray(b_o, np.float32)
    out = np.empty((B, S, D), np.float32)
    for b in range(B):
        acc = results[HG * b]["outT"].astype(np.float32) + results[
            HG * b + 1
        ]["outT"].astype(np.float32)
        out[b] = acc.T + b_o
    return out


def kernel(q, k, v, mask, w_q, b_q, w_k, b_k, w_v, b_v, w_o, b_o, **run_kwargs):
    _import_concourse()
    from concourse.bass_utils import run_bass_kernel_spmd

    nc = build_nc()
    in_maps = make_in_maps(q, k, v, w_q, b_q, w_k, b_k, w_v, b_v, w_o)
    res = run_bass_kernel_spmd(nc, in_maps, core_ids=list(range(NCORES)), **run_kwargs)
    out = gather(res.results, b_o)
    kernel.last_result = res
    return out



# revision 19
# speedup vs baseline: 1.3388x; 1.0447x over previous
# Multi-head attention (B=4, S=2048, D=512, H=8) on 8 Trainium2 cores.
#
# Sharding: core c = (batch b=c//2, head-group g=c%2, 4 heads each). Every core
# runs the identical program (SPMD) on its own slice; partial w_o outputs of the
# two head-groups of a batch are summed on the host (+ b_o).
#
# Device dataflow keeps every activation transposed ([feature, token]) so no
# on-device transposes are needed:
#   QT = w_q_g @ qT + b  (PE, din on partitions)        [256, 2048]
#   KT = (w_k_g/8) @ kT + b/8                            [256, 2048]
#   V  = natural [keys, dout] via lhsT = vT chunks       [2048, 4, 65] (+ones col)
#   scores^T[k, q] = K Q^T  (lhsT = KT slice)            per (qc=512, kc=128)
#   E^T = exp(scores^T + causal window mask)             ACT, merged head-pair
#   ctx^T/l = V_aug^T E^T   (m=65: row 64 = l[q])        PSUM accumulate over kc
#   out^T_partial = w_oT_g ctx^T                         [512, 2048] -> DRAM
#
# All matmul operands are float32r (full-rate fp32 on the PE; producers round
# on write). Resident tensors are split into per-512-chunk tiles so phases
# overlap, and each q-chunk's output projection is emitted inside the next
# chunk's attention loop to keep the PE stream dense (HAM stays warm).
import os
import sys

import numpy as np

B, S, D, H = 4, 2048, 512, 8
DK = D // H          # 64
P = 128
NCORES = 8
HG = 2               # head groups (cores per batch)
DH = D // HG         # 256 out dims per core
LH = H // HG         # 4 local heads
QCS = 512            # q/key chunk size
NQC = S // QCS       # 4
KCS = 128            # key tile size (scores psum partition dim)
NEG = -1e9

# "bf16" (half-width operands: FWL weight loads, single-XBUS moving reads),
# "f32r" (replicated fp32, 1 cyc/row at free>=256) or "f32" (exact, 4 cyc/row)
MM_DT = os.environ.get("KERNEL_MM_DT", "bf16")

_CACHE = {}


def _import_concourse():
    for p in ("/opt/trn_rl_repo", "/root/.axon_site/_ro/trn_rl_repo"):
        if os.path.isdir(p) and p not in sys.path:
            sys.path.append(p)
    import concourse.bass as bass          # noqa: F401
    import concourse.mybir as mybir        # noqa: F401
    import concourse.tile as tile          # noqa: F401
    from concourse import bacc             # noqa: F401
    return bass, mybir, tile


def build_nc():
    """Build the (single, shared-by-all-cores) Bass program."""
    if "nc" in _CACHE:
        return _CACHE["nc"]
    bass, mybir, tile = _import_concourse()
    from concourse import bacc
    from contextlib import ExitStack

    f32 = mybir.dt.float32
    if MM_DT == "bf16":
        fr = mybir.dt.bfloat16
    elif MM_DT == "f32r":
        fr = mybir.dt.float32r
    else:
        fr = f32
    Exp = mybir.ActivationFunctionType.Exp

    nc = bacc.Bacc("TRN2", target_bir_lowering=False, debug=False)

    qT = nc.dram_tensor("qT", [D, S], fr, kind="ExternalInput").ap()
    kT = nc.dram_tensor("kT", [D, S], fr, kind="ExternalInput").ap()
    vT = nc.dram_tensor("vT", [D, S], fr, kind="ExternalInput").ap()
    wqT = nc.dram_tensor("wqT", [D, DH], fr, kind="ExternalInput").ap()
    wkT = nc.dram_tensor("wkT", [D, DH], fr, kind="ExternalInput").ap()
    wvT = nc.dram_tensor("wvT", [D, DH], fr, kind="ExternalInput").ap()
    woT = nc.dram_tensor("woT", [DH, D], fr, kind="ExternalInput").ap()
    bq = nc.dram_tensor("bq", [2, P], f32, kind="ExternalInput").ap()
    bk = nc.dram_tensor("bk", [2, P], f32, kind="ExternalInput").ap()
    bv = nc.dram_tensor("bv", [P, DH], f32, kind="ExternalInput").ap()
    mtri = nc.dram_tensor("mtri", [P, P], f32, kind="ExternalInput").ap()
    onesd = nc.dram_tensor("onesd", [P, DK], f32, kind="ExternalInput").ap()
    outT = nc.dram_tensor("outT", [D, S], fr, kind="ExternalOutput").ap()

    with tile.TileContext(nc) as tc, ExitStack() as ctx:
        wpool = ctx.enter_context(tc.tile_pool(name="weights", bufs=1))
        res = ctx.enter_context(tc.tile_pool(name="resident", bufs=1))
        opool = ctx.enter_context(tc.tile_pool(name="ost", bufs=4))

        # Input DMA issue is split across the two HWDGE rings: the SP queue
        # carries only the prologue-critical Q path (and later the small
        # latency-critical l-chain DMAs); everything else issues from the
        # Activation queue, which is idle until the first exp.
        wq_sb = wpool.tile([P, 4, DH], fr, tag="wq")
        bq_sb = wpool.tile([P, 2], f32, tag="bq")
        wk_sb = wpool.tile([P, 4, DH], fr, tag="wk")
        bk_sb = wpool.tile([P, 2], f32, tag="bk")
        wv_sb = wpool.tile([P, 4, DH], fr, tag="wv")
        bv_sb = wpool.tile([P, DH], f32, tag="bv")
        mt_sb = wpool.tile([P, P], f32, tag="mtri")
        ones_sb = wpool.tile([P, DK], f32, tag="ones")
        wo_sb = wpool.tile([P, 2, D], fr, tag="wo")

        def emit_weight_dmas():
            nc.sync.dma_start(wq_sb[:], wqT.rearrange("(c p) m -> p c m", p=P))
            nc.sync.dma_start(bq_sb[:], bq.rearrange("c p -> p c"))
            nc.scalar.dma_start(wk_sb[:], wkT.rearrange("(c p) m -> p c m", p=P))
            nc.scalar.dma_start(bk_sb[:], bk.rearrange("c p -> p c"))
            nc.scalar.dma_start(wv_sb[:], wvT.rearrange("(c p) m -> p c m", p=P))
            nc.scalar.dma_start(bv_sb[:], bv[:])
            nc.scalar.dma_start(mt_sb[:], mtri[:])
            nc.scalar.dma_start(ones_sb[:], onesd[:])
            nc.scalar.dma_start(wo_sb[:], woT.rearrange("(c p) m -> p c m", p=P))

        # per-512-chunk resident tiles -> fine-grained cross-phase deps
        QTs = [res.tile([P, 2, QCS], fr, tag=f"QT{i}", name=f"QT{i}") for i in range(NQC)]
        KTs = [res.tile([P, 2, QCS], fr, tag=f"KT{i}", name=f"KT{i}") for i in range(NQC)]
        Vgs = [
            res.tile([P, 4, LH, DK + 1], fr, tag=f"Vg{i}", name=f"Vg{i}")
            for i in range(NQC)
        ]
        CTs = [res.tile([P, 2, QCS], fr, tag=f"CT{i}", name=f"CT{i}") for i in range(NQC)]

        qT_r = qT.rearrange("(c p) f -> p c f", p=P)
        kT_r = kT.rearrange("(c p) f -> p c f", p=P)
        vT_r = vT.rearrange("(c p) f -> p c f", p=P)
        bv_r = bv_sb.rearrange("p (h d) -> p h d", h=LH)
        ones_r = ones_sb[:, 0 : 4 * LH].rearrange("p (a b) -> p a b", a=4)

        # ---- Unified pipeline ----
        # For i in 0..3: project 512-chunk i, then attention for q-chunk i
        # (causal: it only consumes chunks <= i), then the previous chunk's
        # output projection. Projections share the scores PSUM pool so the
        # whole kernel fits the 8 banks and the PE stream never breaks.
        with (
            tc.tile_pool(name="inq", bufs=2) as qpool,
            tc.tile_pool(name="ink", bufs=2) as kpool,
            tc.tile_pool(name="inv", bufs=2) as vpool,
            tc.tile_pool(name="et", bufs=8) as epool,
            tc.tile_pool(name="sc", bufs=2, space="PSUM") as scp,
            tc.tile_pool(name="cx", bufs=2, space="PSUM") as cxp,
            tc.tile_pool(name="ls", bufs=1) as lpool,
            tc.tile_pool(name="cbst", bufs=2) as cbpool,
        ):

            # Projections, output projections and attention all share one PE
            # instruction stream: proj/oproj matmul groups are interleaved as
            # "fillers" between attention iterations. A filler allocates from
            # the same "sc" PSUM rotation as the score tiles, so it only ever
            # waits on an exp already in flight (never on anything behind it
            # in the in-order PE queue). The PE therefore never has a
            # low-duty window and the HAM clock-gate stays at 2.4 GHz.
            def alloc_chunk(fc):
                qch = qpool.tile([P, 4, QCS], fr, tag="qch", name=f"qch{fc}")
                kch = kpool.tile([P, 4, QCS], fr, tag="kch", name=f"kch{fc}")
                vch = vpool.tile([P, 4, QCS], fr, tag="vch", name=f"vch{fc}")
                return qch, kch, vch

            def emit_fetch(fc, tiles, kv_eng=None):
                qch, kch, vch = tiles
                kv = kv_eng if kv_eng is not None else nc.sync
                sl = slice(fc * QCS, (fc + 1) * QCS)
                nc.sync.dma_start(qch[:], qT_r[:, :, sl])
                kv.dma_start(kch[:], kT_r[:, :, sl])
                kv.dma_start(vch[:], vT_r[:, :, sl])

            def proj_groups(fc, qch, kch, vch):
                def g_q():
                    ps = scp.tile([P, 2, QCS], f32, tag="sc", bufs=3, name=f"psq{fc}")
                    for mo in range(2):
                        for c in range(4):
                            nc.tensor.matmul(
                                ps[:, mo, :], wq_sb[:, c, mo * P : (mo + 1) * P],
                                qch[:, c, :], start=(c == 0), stop=(c == 3),
                            )
                    for mo in range(2):
                        nc.vector.tensor_add(
                            QTs[fc][:, mo, :], ps[:, mo, :],
                            bq_sb[:, mo : mo + 1].to_broadcast((P, QCS)),
                        )

                def g_k():
                    ps = scp.tile([P, 2, QCS], f32, tag="sc", bufs=3, name=f"psk{fc}")
                    for mo in range(2):
                        for c in range(4):
                            nc.tensor.matmul(
                                ps[:, mo, :], wk_sb[:, c, mo * P : (mo + 1) * P],
                                kch[:, c, :], start=(c == 0), stop=(c == 3),
                            )
                    for mo in range(2):
                        nc.vector.tensor_add(
                            KTs[fc][:, mo, :], ps[:, mo, :],
                            bk_sb[:, mo : mo + 1].to_broadcast((P, QCS)),
                        )

                def g_v(k2):
                    if k2 == 0:
                        nc.vector.tensor_copy(Vgs[fc][:, :, :, DK], ones_r)
                    ps = scp.tile([P, 2, QCS], f32, tag="sc", bufs=3, name=f"psv{fc}{k2}")
                    for kl2 in range(2):
                        kl = k2 * 2 + kl2
                        for c in range(4):
                            nc.tensor.matmul(
                                ps[:, kl2, 0:DH],
                                vch[:, c, kl * P : (kl + 1) * P], wv_sb[:, c, :],
                                start=(c == 0), stop=(c == 3),
                            )
                    for kl2 in range(2):
                        kl = k2 * 2 + kl2
                        nc.vector.tensor_add(
                            Vgs[fc][:, kl, :, 0:DK],
                            ps[:, kl2, 0:DH].rearrange("p (h d) -> p h d", h=LH),
                            bv_r,
                        )

                return [g_q, g_k, lambda: g_v(0), lambda: g_v(1)]

            def oproj_groups(qc):
                qsl = slice(qc * QCS, (qc + 1) * QCS)

                def g_o(half):
                    ps = scp.tile([P, 2, QCS], f32, tag="sc", bufs=3, name=f"pso{qc}{half}")
                    for m2 in range(2):
                        mo = half * 2 + m2
                        msl = slice(mo * P, (mo + 1) * P)
                        for c in range(2):
                            nc.tensor.matmul(
                                ps[:, m2, :], wo_sb[:, c, msl], CTs[qc][:, c, :],
                                start=(c == 0), stop=(c == 1),
                            )
                    for m2 in range(2):
                        mo = half * 2 + m2
                        msl = slice(mo * P, (mo + 1) * P)
                        ost = opool.tile(
                            [P, QCS], fr, tag="ost", name=f"ost{qc}{mo}"
                        )
                        nc.vector.tensor_copy(ost[:], ps[:, m2, :])
                        nc.sync.dma_start(outT[msl, qsl], ost[:])

                return [lambda: g_o(0), lambda: g_o(1)]

            def attn(qc, fillers):
                # Lookahead-2 software pipeline: the PE queue runs
                #   s0 s1 s2 [fillers] c0 s3 c1 s4 c2 ... c(n-1)
                # while ACT runs e0 e1 e2 ... back-to-back. ctx(k) waits on
                # exp(k), but the scores for k+2 are already AHEAD of it in
                # the queue, so the ACT stream (the critical resource) never
                # gaps. Score PSUM is triple-buffered; the ctx accumulators
                # are single-buffered and their pair-boundary drain (l-chain)
                # is bridged by the filler groups emitted just before c0.
                nkc = (qc + 1) * (QCS // KCS)

                def emit_scores(pr, kc):
                    fc, kk = kc // 4, kc % 4
                    ksl = slice(kk * KCS, (kk + 1) * KCS)
                    d = kc * KCS - qc * QCS
                    lo = max(d, 0)
                    sct = scp.tile([P, 2, QCS], f32, tag="sc", bufs=3)
                    nc.tensor.matmul(
                        sct[:, 0, lo:QCS], KTs[fc][0:DK, pr, ksl],
                        QTs[qc][0:DK, pr, lo:QCS], start=True, stop=True,
                    )
                    nc.tensor.matmul(
                        sct[:, 1, lo:QCS], KTs[fc][DK:P, pr, ksl],
                        QTs[qc][DK:P, pr, lo:QCS], start=True, stop=True,
                    )
                    if d >= 0:  # diagonal tile: causal window mask
                        nc.vector.tensor_add(
                            sct[:, 0, d : d + P], sct[:, 0, d : d + P], mt_sb[:]
                        )
                        nc.vector.tensor_add(
                            sct[:, 1, d : d + P], sct[:, 1, d : d + P], mt_sb[:]
                        )
                    return sct, lo

                def emit_exp(sct, lo):
                    et = epool.tile([P, 2, QCS], fr, tag="et")
                    nc.scalar.activation(et[:, :, lo:QCS], sct[:, :, lo:QCS], Exp)
                    return et

                def emit_ctx(cA, cB, pr, kc, et, lo):
                    fc, kk = kc // 4, kc % 4
                    first, last = kc == 0, kc == nkc - 1
                    nc.tensor.matmul(
                        cA[0 : DK + 1, lo:QCS], Vgs[fc][:, kk, 2 * pr + 0, :],
                        et[:, 0, lo:QCS], start=first, stop=last,
                    )
                    nc.tensor.matmul(
                        cB[0 : DK + 1, lo:QCS], Vgs[fc][:, kk, 2 * pr + 1, :],
                        et[:, 1, lo:QCS], start=first, stop=last,
                    )

                # Fillers pop one per iteration slot (kc>=2) so they never
                # starve the ACT stream; ~5 exps of prime cover the previous
                # pair's l-chain drain before c0 blocks the in-order queue.
                PRIME = min(5, nkc - 1)
                fq = list(fillers)

                for pr in range(2):
                    cA = cxp.tile([P, QCS], f32, tag="cx0", bufs=1, name=f"cA{qc}{pr}")
                    cB = cxp.tile([P, QCS], f32, tag="cx1", bufs=1, name=f"cB{qc}{pr}")
                    pipe = []  # (sct, lo, et)
                    cc = 0
                    for kc in range(nkc):
                        s, lo = emit_scores(pr, kc)
                        pipe.append([s, lo, emit_exp(s, lo)])
                        if fq and kc >= 2:
                            fq.pop(0)()
                        if kc >= PRIME:
                            emit_ctx(cA, cB, pr, cc, pipe[cc][2], pipe[cc][1])
                            cc += 1
                            if cc <= kc - 2:
                                emit_ctx(cA, cB, pr, cc, pipe[cc][2], pipe[cc][1])
                                cc += 1
                    while cc < nkc:
                        emit_ctx(cA, cB, pr, cc, pipe[cc][2], pipe[cc][1])
                        cc += 1
                    if pr == 1:
                        while fq:  # drain any leftover filler groups
                            fq.pop(0)()
                    # l rows sit on PSUM partition 64. Per head: DVE copy to
                    # SBUF, DMA the row to partition 0, reciprocal, GPSIMD
                    # partition_broadcast, multiply — split per head so each
                    # accumulator frees as early as possible.
                    lr = lpool.tile([DK + 1, 2, QCS], f32, tag="lr")
                    l0 = lpool.tile([1, 2, QCS], f32, tag="l0")
                    r0 = lpool.tile([1, 2, QCS], f32, tag="r0")
                    nc.vector.tensor_copy(lr[DK : DK + 1, 0, :], cA[DK : DK + 1, :])
                    nc.sync.dma_start(l0[:, 0, :], lr[DK : DK + 1, 0, :])
                    nc.vector.tensor_copy(lr[DK : DK + 1, 1, :], cB[DK : DK + 1, :])
                    nc.sync.dma_start(l0[:, 1, :], lr[DK : DK + 1, 1, :])
                    nc.vector.reciprocal_approx_fast(r0[:, 0, :], l0[:, 0, :])
                    rbAs = cbpool.tile([DK, QCS], f32, tag="rbAs")
                    nc.gpsimd.partition_broadcast(rbAs[:], r0[0:1, 0, :], channels=DK)
                    nc.vector.reciprocal_approx_fast(r0[:, 1, :], l0[:, 1, :])
                    rbBs = cbpool.tile([DK, QCS], f32, tag="rbBs")
                    nc.gpsimd.partition_broadcast(rbBs[:], r0[0:1, 1, :], channels=DK)
                    nc.vector.tensor_mul(CTs[qc][0:DK, pr, :], cA[0:DK, :], rbAs[:])
                    cbs = cbpool.tile([DK, QCS], fr, tag="cbs")
                    nc.vector.tensor_mul(cbs[:], cB[0:DK, :], rbBs[:])
                    # DMA moves it down to partitions 64..127 (DVE can't shift
                    # partitions; DMA can't read PSUM - hence the SBUF hop)
                    nc.sync.dma_start(CTs[qc][DK:P, pr, :], cbs[:])

            # Prologue: chunk 0 projection standalone (doubles as HAM warmup).
            ch0 = alloc_chunk(0)
            emit_fetch(0, ch0, kv_eng=nc.scalar)
            emit_weight_dmas()
            for g in proj_groups(0, *ch0):
                g()
            # attn(i) carries oproj(i-1) + proj(i+1) as PE filler groups; the
            # chunk i+1 prefetch rides as the first "filler" so its bulk DMA
            # issue never sits ahead of an l-chain DMA on the SP queue.
            # Chunk 3's K/V groups ride inside attn(3) itself (they are only
            # needed from kc=12) so attn(3)'s pair boundaries have cover too.
            gq3 = gk3 = gv30 = gv31 = None
            for i in range(NQC):
                fillers = []
                if i + 1 < NQC:
                    ch = alloc_chunk(i + 1)
                    fillers += [
                        (lambda fc, t: lambda: emit_fetch(fc, t))(i + 1, ch)
                    ]
                if i >= 1:
                    fillers += oproj_groups(i - 1)
                if i + 1 < NQC:
                    gs = proj_groups(i + 1, *ch)
                    if i + 1 == 3:
                        gq3, gk3, gv30, gv31 = gs
                        fillers += [gq3]
                    else:
                        fillers += gs
                elif i == NQC - 1:
                    fillers += [gk3, gv30, gv31]
                attn(i, fillers)
            for g in oproj_groups(NQC - 1):
                g()

    nc.compile()
    _CACHE["nc"] = nc
    return nc


def make_in_maps(q, k, v, w_q, b_q, w_k, b_k, w_v, b_v, w_o):
    """Host-side sharding: per-core input dict (all fp32, C-contiguous)."""
    f = np.float32
    q = np.asarray(q, f)
    k = np.asarray(k, f)
    v = np.asarray(v, f)
    w_q = np.asarray(w_q, f)
    w_k = np.asarray(w_k, f)
    w_v = np.asarray(w_v, f)
    w_o = np.asarray(w_o, f)
    b_q = np.asarray(b_q, f)
    b_k = np.asarray(b_k, f)
    b_v = np.asarray(b_v, f)

    if MM_DT == "bf16":
        import ml_dtypes

        mdt = ml_dtypes.bfloat16
    else:
        mdt = f

    scale = np.float32(1.0 / np.sqrt(DK))
    qTl = [np.ascontiguousarray(q[b].T.astype(mdt)) for b in range(B)]
    kTl = [np.ascontiguousarray(k[b].T.astype(mdt)) for b in range(B)]
    vTl = [np.ascontiguousarray(v[b].T.astype(mdt)) for b in range(B)]
    ii = np.arange(P)
    mtri = np.where(ii[:, None] > ii[None, :], f(NEG), f(0.0)).astype(f)

    per_g = []
    for g in range(HG):
        gsl = slice(g * DH, (g + 1) * DH)
        per_g.append(
            dict(
                wqT=np.ascontiguousarray(w_q[gsl, :].T.astype(mdt)),
                wkT=np.ascontiguousarray((w_k[gsl, :].T * scale).astype(mdt)),
                wvT=np.ascontiguousarray(w_v[gsl, :].T.astype(mdt)),
                woT=np.ascontiguousarray(w_o[:, gsl].T.astype(mdt)),
                bq=np.ascontiguousarray(b_q[gsl].reshape(2, P)),
                bk=np.ascontiguousarray((b_k[gsl] * scale).reshape(2, P)),
                bv=np.ascontiguousarray(np.broadcast_to(b_v[gsl], (P, DH))),
                mtri=mtri,
                onesd=np.ones((P, DK), f),
            )
        )

    in_maps = []
    for c in range(NCORES):
        b, g = c // HG, c % HG
        m = dict(qT=qTl[b], kT=kTl[b], vT=vTl[b], **per_g[g])
        in_maps.append(m)
    return in_maps


def gather(results, b_o):
    """Sum head-group partials per batch, un-transpose, add b_o."""
    b_o = np.asarray(b_o, np.float32)
    out = np.empty((B, S, D), np.float32)
    for b in range(B):
        acc = results[HG * b]["outT"].astype(np.float32) + results[
            HG * b + 1
        ]["outT"].astype(np.float32)
        out[b] = acc.T + b_o
    return out


def kernel(q, k, v, mask, w_q, b_q, w_k, b_k, w_v, b_v, w_o, b_o, **run_kwargs):
    _import_concourse()
    from concourse.bass_utils import run_bass_kernel_spmd

    nc = build_nc()
    in_maps = make_in_maps(q, k, v, w_q, b_q, w_k, b_k, w_v, b_v, w_o)
    res = run_bass_kernel_spmd(nc, in_maps, core_ids=list(range(NCORES)), **run_kwargs)
    out = gather(res.results, b_o)
    kernel.last_result = res
    return out



# revision 24
# speedup vs baseline: 1.3643x; 1.0191x over previous
# Multi-head attention (B=4, S=2048, D=512, H=8) on 8 Trainium2 cores.
#
# Sharding: core c = (batch b=c//2, head-group g=c%2, 4 heads each). Every core
# runs the identical program (SPMD) on its own slice; partial w_o outputs of the
# two head-groups of a batch are summed on the host (+ b_o).
#
# Device dataflow keeps every activation transposed ([feature, token]) so no
# on-device transposes are needed:
#   QT = w_q_g @ qT + b  (PE, din on partitions)        [256, 2048]
#   KT = (w_k_g/8) @ kT + b/8                            [256, 2048]
#   V  = natural [keys, dout] via lhsT = vT chunks       [2048, 4, 65] (+ones col)
#   scores^T[k, q] = K Q^T  (lhsT = KT slice)            per (qc=512, kc=128)
#   E^T = exp(scores^T + causal window mask)             ACT, merged head-pair
#   ctx^T/l = V_aug^T E^T   (m=65: row 64 = l[q])        PSUM accumulate over kc
#   out^T_partial = w_oT_g ctx^T                         [512, 2048] -> DRAM
#
# All matmul operands are float32r (full-rate fp32 on the PE; producers round
# on write). Resident tensors are split into per-512-chunk tiles so phases
# overlap, and each q-chunk's output projection is emitted inside the next
# chunk's attention loop to keep the PE stream dense (HAM stays warm).
import os
import sys

import numpy as np

B, S, D, H = 4, 2048, 512, 8
DK = D // H          # 64
P = 128
NCORES = 8
HG = 2               # head groups (cores per batch)
DH = D // HG         # 256 out dims per core
LH = H // HG         # 4 local heads
QCS = 512            # q/key chunk size
NQC = S // QCS       # 4
KCS = 128            # key tile size (scores psum partition dim)
NEG = -1e9

# "bf16" (half-width operands: FWL weight loads, single-XBUS moving reads),
# "f32r" (replicated fp32, 1 cyc/row at free>=256) or "f32" (exact, 4 cyc/row)
MM_DT = os.environ.get("KERNEL_MM_DT", "bf16")

_CACHE = {}


def _import_concourse():
    for p in ("/opt/trn_rl_repo", "/root/.axon_site/_ro/trn_rl_repo"):
        if os.path.isdir(p) and p not in sys.path:
            sys.path.append(p)
    import concourse.bass as bass          # noqa: F401
    import concourse.mybir as mybir        # noqa: F401
    import concourse.tile as tile          # noqa: F401
    from concourse import bacc             # noqa: F401
    return bass, mybir, tile


def build_nc():
    """Build the (single, shared-by-all-cores) Bass program."""
    if "nc" in _CACHE:
        return _CACHE["nc"]
    bass, mybir, tile = _import_concourse()
    from concourse import bacc
    from contextlib import ExitStack

    f32 = mybir.dt.float32
    if MM_DT == "bf16":
        fr = mybir.dt.bfloat16
    elif MM_DT == "f32r":
        fr = mybir.dt.float32r
    else:
        fr = f32
    Exp = mybir.ActivationFunctionType.Exp

    nc = bacc.Bacc("TRN2", target_bir_lowering=False, debug=False)

    qT = nc.dram_tensor("qT", [D, S], fr, kind="ExternalInput").ap()
    kT = nc.dram_tensor("kT", [D, S], fr, kind="ExternalInput").ap()
    vT = nc.dram_tensor("vT", [D, S], fr, kind="ExternalInput").ap()
    wqT = nc.dram_tensor("wqT", [D, DH], fr, kind="ExternalInput").ap()
    wkT = nc.dram_tensor("wkT", [D, DH], fr, kind="ExternalInput").ap()
    wvT = nc.dram_tensor("wvT", [D, DH], fr, kind="ExternalInput").ap()
    woT = nc.dram_tensor("woT", [DH, D], fr, kind="ExternalInput").ap()
    bq = nc.dram_tensor("bq", [2, P], f32, kind="ExternalInput").ap()
    bk = nc.dram_tensor("bk", [2, P], f32, kind="ExternalInput").ap()
    bv = nc.dram_tensor("bv", [P, DH], f32, kind="ExternalInput").ap()
    mtri = nc.dram_tensor("mtri", [P, P], f32, kind="ExternalInput").ap()
    onesd = nc.dram_tensor("onesd", [P, DK], f32, kind="ExternalInput").ap()
    outT = nc.dram_tensor("outT", [D, S], fr, kind="ExternalOutput").ap()

    with tile.TileContext(nc) as tc, ExitStack() as ctx:
        wpool = ctx.enter_context(tc.tile_pool(name="weights", bufs=1))
        res = ctx.enter_context(tc.tile_pool(name="resident", bufs=1))
        opool = ctx.enter_context(tc.tile_pool(name="ost", bufs=4))

        # Input DMA issue is split across the two HWDGE rings: the SP queue
        # carries only the prologue-critical Q path (and later the small
        # latency-critical l-chain DMAs); everything else issues from the
        # Activation queue, which is idle until the first exp.
        wq_sb = wpool.tile([P, 4, DH], fr, tag="wq")
        bq_sb = wpool.tile([P, 2], f32, tag="bq")
        wk_sb = wpool.tile([P, 4, DH], fr, tag="wk")
        bk_sb = wpool.tile([P, 2], f32, tag="bk")
        wv_sb = wpool.tile([P, 4, DH], fr, tag="wv")
        bv_sb = wpool.tile([P, DH], f32, tag="bv")
        mt_sb = wpool.tile([P, P], f32, tag="mtri")
        ones_sb = wpool.tile([P, DK], f32, tag="ones")
        wo_sb = wpool.tile([P, 2, D], fr, tag="wo")

        def emit_weight_dmas():
            wq_r = wqT.rearrange("(c p) m -> p c m", p=P)
            nc.sync.dma_start(wq_sb[:, 0, :], wq_r[:, 0, :])
            nc.sync.dma_start(wq_sb[:, 1:4, :], wq_r[:, 1:4, :])
            nc.sync.dma_start(bq_sb[:], bq.rearrange("c p -> p c"))
            nc.scalar.dma_start(wk_sb[:], wkT.rearrange("(c p) m -> p c m", p=P))
            nc.scalar.dma_start(bk_sb[:], bk.rearrange("c p -> p c"))
            nc.scalar.dma_start(wv_sb[:], wvT.rearrange("(c p) m -> p c m", p=P))
            nc.scalar.dma_start(bv_sb[:], bv[:])
            nc.scalar.dma_start(mt_sb[:], mtri[:])
            nc.scalar.dma_start(ones_sb[:], onesd[:])
            nc.scalar.dma_start(wo_sb[:], woT.rearrange("(c p) m -> p c m", p=P))

        # per-512-chunk resident tiles -> fine-grained cross-phase deps
        QTs = [res.tile([P, 2, QCS], fr, tag=f"QT{i}", name=f"QT{i}") for i in range(NQC)]
        KTs = [res.tile([P, 2, QCS], fr, tag=f"KT{i}", name=f"KT{i}") for i in range(NQC)]
        Vgs = [
            res.tile([P, 4, LH, DK + 1], fr, tag=f"Vg{i}", name=f"Vg{i}")
            for i in range(NQC)
        ]
        CTs = [res.tile([P, 2, QCS], fr, tag=f"CT{i}", name=f"CT{i}") for i in range(NQC)]

        qT_r = qT.rearrange("(c p) f -> p c f", p=P)
        kT_r = kT.rearrange("(c p) f -> p c f", p=P)
        vT_r = vT.rearrange("(c p) f -> p c f", p=P)
        bv_r = bv_sb.rearrange("p (h d) -> p h d", h=LH)
        ones_r = ones_sb[:, 0 : 4 * LH].rearrange("p (a b) -> p a b", a=4)

        # ---- Unified pipeline ----
        # For i in 0..3: project 512-chunk i, then attention for q-chunk i
        # (causal: it only consumes chunks <= i), then the previous chunk's
        # output projection. Projections share the scores PSUM pool so the
        # whole kernel fits the 8 banks and the PE stream never breaks.
        with (
            tc.tile_pool(name="inq", bufs=2) as qpool,
            tc.tile_pool(name="ink", bufs=2) as kpool,
            tc.tile_pool(name="inv", bufs=2) as vpool,
            tc.tile_pool(name="et", bufs=8) as epool,
            tc.tile_pool(name="sc", bufs=2, space="PSUM") as scp,
            tc.tile_pool(name="cx", bufs=2, space="PSUM") as cxp,
            tc.tile_pool(name="ls", bufs=1) as lpool,
            tc.tile_pool(name="cbst", bufs=2) as cbpool,
        ):

            # Projections, output projections and attention all share one PE
            # instruction stream: proj/oproj matmul groups are interleaved as
            # "fillers" between attention iterations. A filler allocates from
            # the same "sc" PSUM rotation as the score tiles, so it only ever
            # waits on an exp already in flight (never on anything behind it
            # in the in-order PE queue). The PE therefore never has a
            # low-duty window and the HAM clock-gate stays at 2.4 GHz.
            def alloc_chunk(fc):
                qch = qpool.tile([P, 4, QCS], fr, tag="qch", name=f"qch{fc}")
                kch = kpool.tile([P, 4, QCS], fr, tag="kch", name=f"kch{fc}")
                vch = vpool.tile([P, 4, QCS], fr, tag="vch", name=f"vch{fc}")
                return qch, kch, vch

            def emit_fetch(fc, tiles, kv_eng=None):
                qch, kch, vch = tiles
                kv = kv_eng if kv_eng is not None else nc.sync
                sl = slice(fc * QCS, (fc + 1) * QCS)
                if fc == 0:
                    # split so the first projection matmul (which reads the
                    # c=0 slice) can start as early as possible
                    nc.sync.dma_start(qch[:, 0, :], qT_r[:, 0, sl])
                    nc.sync.dma_start(qch[:, 1:4, :], qT_r[:, 1:4, sl])
                else:
                    nc.sync.dma_start(qch[:], qT_r[:, :, sl])
                kv.dma_start(kch[:], kT_r[:, :, sl])
                kv.dma_start(vch[:], vT_r[:, :, sl])

            def proj_groups(fc, qch, kch, vch):
                def g_q():
                    ps = scp.tile([P, 2, QCS], f32, tag="sc", bufs=3, name=f"psq{fc}")
                    for mo in range(2):
                        for c in range(4):
                            nc.tensor.matmul(
                                ps[:, mo, :], wq_sb[:, c, mo * P : (mo + 1) * P],
                                qch[:, c, :], start=(c == 0), stop=(c == 3),
                            )
                    for mo in range(2):
                        nc.vector.tensor_add(
                            QTs[fc][:, mo, :], ps[:, mo, :],
                            bq_sb[:, mo : mo + 1].to_broadcast((P, QCS)),
                        )

                def g_k():
                    ps = scp.tile([P, 2, QCS], f32, tag="sc", bufs=3, name=f"psk{fc}")
                    for mo in range(2):
                        for c in range(4):
                            nc.tensor.matmul(
                                ps[:, mo, :], wk_sb[:, c, mo * P : (mo + 1) * P],
                                kch[:, c, :], start=(c == 0), stop=(c == 3),
                            )
                    for mo in range(2):
                        nc.vector.tensor_add(
                            KTs[fc][:, mo, :], ps[:, mo, :],
                            bk_sb[:, mo : mo + 1].to_broadcast((P, QCS)),
                        )

                def g_v(k2):
                    if k2 == 0:
                        nc.vector.tensor_copy(Vgs[fc][:, :, :, DK], ones_r)
                    ps = scp.tile([P, 2, QCS], f32, tag="sc", bufs=3, name=f"psv{fc}{k2}")
                    for kl2 in range(2):
                        kl = k2 * 2 + kl2
                        for c in range(4):
                            nc.tensor.matmul(
                                ps[:, kl2, 0:DH],
                                vch[:, c, kl * P : (kl + 1) * P], wv_sb[:, c, :],
                                start=(c == 0), stop=(c == 3),
                            )
                    for kl2 in range(2):
                        kl = k2 * 2 + kl2
                        nc.vector.tensor_add(
                            Vgs[fc][:, kl, :, 0:DK],
                            ps[:, kl2, 0:DH].rearrange("p (h d) -> p h d", h=LH),
                            bv_r,
                        )

                return [g_q, g_k, lambda: g_v(0), lambda: g_v(1)]

            def oproj_groups(qc):
                qsl = slice(qc * QCS, (qc + 1) * QCS)

                def g_o(half):
                    ps = scp.tile([P, 2, QCS], f32, tag="sc", bufs=3, name=f"pso{qc}{half}")
                    for m2 in range(2):
                        mo = half * 2 + m2
                        msl = slice(mo * P, (mo + 1) * P)
                        for c in range(2):
                            nc.tensor.matmul(
                                ps[:, m2, :], wo_sb[:, c, msl], CTs[qc][:, c, :],
                                start=(c == 0), stop=(c == 1),
                            )
                    for m2 in range(2):
                        mo = half * 2 + m2
                        msl = slice(mo * P, (mo + 1) * P)
                        ost = opool.tile(
                            [P, QCS], fr, tag="ost", name=f"ost{qc}{mo}"
                        )
                        nc.vector.tensor_copy(ost[:], ps[:, m2, :])
                        nc.sync.dma_start(outT[msl, qsl], ost[:])

                return [lambda: g_o(0), lambda: g_o(1)]

            # ---- Flat cross-boundary software pipeline ----
            # One global scores/exp cursor runs exactly two (qc, pr, kc) items
            # ahead of a ctx cursor, across pair AND phase boundaries, so the
            # ACT exp stream (the critical resource) never gaps: while pair
            # p's last ctx pairs and l-chain drain, pair p+1's scores already
            # stream on the PE. Score PSUM is triple-buffered; the ctx
            # accumulators are single-buffered (their WAR on the previous
            # pair's l-chain multiply resolves under ~4 exps of buffered ACT
            # work). Proj/oproj/fetch filler groups pop one per item.
            def nkc_of(qc):
                return (qc + 1) * (QCS // KCS)

            def emit_scores(qc, pr, kc):
                fc, kk = kc // 4, kc % 4
                ksl = slice(kk * KCS, (kk + 1) * KCS)
                d = kc * KCS - qc * QCS
                lo = max(d, 0)
                sct = scp.tile([P, 2, QCS], f32, tag="sc", bufs=3)
                nc.tensor.matmul(
                    sct[:, 0, lo:QCS], KTs[fc][0:DK, pr, ksl],
                    QTs[qc][0:DK, pr, lo:QCS], start=True, stop=True,
                )
                nc.tensor.matmul(
                    sct[:, 1, lo:QCS], KTs[fc][DK:P, pr, ksl],
                    QTs[qc][DK:P, pr, lo:QCS], start=True, stop=True,
                )
                if d >= 0:  # diagonal tile: causal window mask
                    nc.vector.tensor_add(
                        sct[:, 0, d : d + P], sct[:, 0, d : d + P], mt_sb[:]
                    )
                    nc.vector.tensor_add(
                        sct[:, 1, d : d + P], sct[:, 1, d : d + P], mt_sb[:]
                    )
                et = epool.tile([P, 2, QCS], fr, tag="et")
                nc.scalar.activation(et[:, :, lo:QCS], sct[:, :, lo:QCS], Exp)
                return et, lo

            def emit_ctx(cx, qc, pr, kc, et, lo):
                fc, kk = kc // 4, kc % 4
                cA, cB = cx
                first, last = kc == 0, kc == nkc_of(qc) - 1
                nc.tensor.matmul(
                    cA[0 : DK + 1, lo:QCS], Vgs[fc][:, kk, 2 * pr + 0, :],
                    et[:, 0, lo:QCS], start=first, stop=last,
                )
                nc.tensor.matmul(
                    cB[0 : DK + 1, lo:QCS], Vgs[fc][:, kk, 2 * pr + 1, :],
                    et[:, 1, lo:QCS], start=first, stop=last,
                )

            def emit_lchain(cx, qc, pr):
                # l rows sit on PSUM partition 64. Per head: DVE copy to
                # SBUF, DMA the row to partition 0, reciprocal there, GPSIMD
                # partition_broadcast, multiply — split per head so each
                # accumulator frees as early as possible.
                cA, cB = cx
                lr = lpool.tile([DK + 1, 2, QCS], f32, tag="lr")
                l0 = lpool.tile([1, 2, QCS], f32, tag="l0")
                r0 = lpool.tile([1, 2, QCS], f32, tag="r0")
                nc.vector.tensor_copy(lr[DK : DK + 1, 0, :], cA[DK : DK + 1, :])
                nc.sync.dma_start(l0[:, 0, :], lr[DK : DK + 1, 0, :])
                nc.vector.tensor_copy(lr[DK : DK + 1, 1, :], cB[DK : DK + 1, :])
                nc.sync.dma_start(l0[:, 1, :], lr[DK : DK + 1, 1, :])
                nc.vector.reciprocal_approx_fast(r0[:, 0, :], l0[:, 0, :])
                rbAs = cbpool.tile([DK, QCS], f32, tag="rbAs")
                nc.gpsimd.partition_broadcast(rbAs[:], r0[0:1, 0, :], channels=DK)
                nc.vector.reciprocal_approx_fast(r0[:, 1, :], l0[:, 1, :])
                rbBs = cbpool.tile([DK, QCS], f32, tag="rbBs")
                nc.gpsimd.partition_broadcast(rbBs[:], r0[0:1, 1, :], channels=DK)
                nc.vector.tensor_mul(CTs[qc][0:DK, pr, :], cA[0:DK, :], rbAs[:])
                cbs = cbpool.tile([DK, QCS], fr, tag="cbs")
                nc.vector.tensor_mul(cbs[:], cB[0:DK, :], rbBs[:])
                # DMA moves it down to partitions 64..127 (DVE can't shift
                # partitions; DMA can't read PSUM - hence the SBUF hop)
                nc.sync.dma_start(CTs[qc][DK:P, pr, :], cbs[:])

            # Prologue: fetch chunk 0 + weights; project Q/K standalone (also
            # the HAM warmup). V-groups ride as the first fillers of phase 0.
            ch0 = alloc_chunk(0)
            emit_fetch(0, ch0, kv_eng=nc.scalar)
            emit_weight_dmas()
            pg0 = proj_groups(0, *ch0)
            pg0[0]()  # Q
            pg0[1]()  # K
            phase_fillers = {0: [pg0[2], pg0[3]]}
            gq3 = gk3 = gv30 = gv31 = None
            for i in range(NQC):
                fl = phase_fillers.setdefault(i, [])
                if i + 1 < NQC:
                    ch = alloc_chunk(i + 1)
                    fl.append((lambda fc, t: lambda: emit_fetch(fc, t))(i + 1, ch))
                    gs = proj_groups(i + 1, *ch)
                    if i + 1 == 3:
                        gq3, gk3, gv30, gv31 = gs
                        fl.append(gq3)
                    else:
                        fl.extend(gs)
                elif i == NQC - 1:
                    fl.extend([gk3, gv30, gv31])

            items = [
                (qc, pr, kc)
                for qc in range(NQC)
                for pr in range(2)
                for kc in range(nkc_of(qc))
            ]
            LOOK = 2
            pipes = {}  # (qc, pr) -> list of (et, lo)
            cxs = {}    # (qc, pr) -> (cA, cB)
            fq = []
            C = 0

            def advance_ctx():
                nonlocal C
                qc, pr, kc = items[C]
                if kc == 0:
                    cxs[(qc, pr)] = (
                        cxp.tile([P, QCS], f32, tag="cx0", bufs=1,
                                 name=f"cA{qc}{pr}"),
                        cxp.tile([P, QCS], f32, tag="cx1", bufs=1,
                                 name=f"cB{qc}{pr}"),
                    )
                et, lo = pipes[(qc, pr)][kc]
                emit_ctx(cxs[(qc, pr)], qc, pr, kc, et, lo)
                pipes[(qc, pr)][kc] = None
                if kc == nkc_of(qc) - 1:
                    emit_lchain(cxs[(qc, pr)], qc, pr)
                    if pr == 1:
                        # CTs[qc] writers are now all emitted; oproj(qc) may
                        # safely enter the filler queue
                        fq.extend(oproj_groups(qc))
                C += 1

            for si, (qc, pr, kc) in enumerate(items):
                if pr == 0 and kc == 0:
                    fq.extend(phase_fillers.get(qc, ()))
                pipes.setdefault((qc, pr), [None] * nkc_of(qc))[kc] = emit_scores(
                    qc, pr, kc
                )
                if fq and si >= 2:
                    fq.pop(0)()
                while C <= si - LOOK:
                    advance_ctx()
            while C < len(items):
                advance_ctx()
            while fq:
                fq.pop(0)()

    nc.compile()
    _CACHE["nc"] = nc
    return nc


def make_in_maps(q, k, v, w_q, b_q, w_k, b_k, w_v, b_v, w_o):
    """Host-side sharding: per-core input dict (all fp32, C-contiguous)."""
    f = np.float32
    q = np.asarray(q, f)
    k = np.asarray(k, f)
    v = np.asarray(v, f)
    w_q = np.asarray(w_q, f)
    w_k = np.asarray(w_k, f)
    w_v = np.asarray(w_v, f)
    w_o = np.asarray(w_o, f)
    b_q = np.asarray(b_q, f)
    b_k = np.asarray(b_k, f)
    b_v = np.asarray(b_v, f)

    if MM_DT == "bf16":
        import ml_dtypes

        mdt = ml_dtypes.bfloat16
    else:
        mdt = f

    scale = np.float32(1.0 / np.sqrt(DK))
    qTl = [np.ascontiguousarray(q[b].T.astype(mdt)) for b in range(B)]
    kTl = [np.ascontiguousarray(k[b].T.astype(mdt)) for b in range(B)]
    vTl = [np.ascontiguousarray(v[b].T.astype(mdt)) for b in range(B)]
    ii = np.arange(P)
    mtri = np.where(ii[:, None] > ii[None, :], f(NEG), f(0.0)).astype(f)

    per_g = []
    for g in range(HG):
        gsl = slice(g * DH, (g + 1) * DH)
        per_g.append(
            dict(
                wqT=np.ascontiguousarray(w_q[gsl, :].T.astype(mdt)),
                wkT=np.ascontiguousarray((w_k[gsl, :].T * scale).astype(mdt)),
                wvT=np.ascontiguousarray(w_v[gsl, :].T.astype(mdt)),
                woT=np.ascontiguousarray(w_o[:, gsl].T.astype(mdt)),
                bq=np.ascontiguousarray(b_q[gsl].reshape(2, P)),
                bk=np.ascontiguousarray((b_k[gsl] * scale).reshape(2, P)),
                bv=np.ascontiguousarray(np.broadcast_to(b_v[gsl], (P, DH))),
                mtri=mtri,
                onesd=np.ones((P, DK), f),
            )
        )

    in_maps = []
    for c in range(NCORES):
        b, g = c // HG, c % HG
        m = dict(qT=qTl[b], kT=kTl[b], vT=vTl[b], **per_g[g])
        in_maps.append(m)
    return in_maps


def gather(results, b_o):
    """Sum head-group partials per batch, un-transpose, add b_o."""
    b_o = np.asarray(b_o, np.float32)
    out = np.empty((B, S, D), np.float32)
    for b in range(B):
        acc = results[HG * b]["outT"].astype(np.float32) + results[
            HG * b + 1
        ]["outT"].astype(np.float32)
        out[b] = acc.T + b_o
    return out


def kernel(q, k, v, mask, w_q, b_q, w_k, b_k, w_v, b_v, w_o, b_o, **run_kwargs):
    _import_concourse()
    from concourse.bass_utils import run_bass_kernel_spmd

    nc = build_nc()
    in_maps = make_in_maps(q, k, v, w_q, b_q, w_k, b_k, w_v, b_v, w_o)
    res = run_bass_kernel_spmd(nc, in_maps, core_ids=list(range(NCORES)), **run_kwargs)
    out = gather(res.results, b_o)
    kernel.last_result = res
    return out

